# revision 1
# baseline (speedup 1.0000x reference)
"""Trainium2 Bass kernel for nn_Block_softmoe (dense transformer block, B=4 S=2048 C=256 H=8).

Strategy
--------
Sharding: 8 cores = (batch b, query-half). Each core computes the full block for
1024 query rows of one batch. K/V are computed per-core over that batch's keys
(2x redundant K/V projection; tiny at dim 256). No collectives.

Mask compaction: the key mask (Bernoulli 0/1) is applied on the host by
gathering only the kept key rows (~1024 of 2048). This halves the dominant
cost, the softmax exp on the ScalarE (ACT) engine, plus the score/attnV
matmuls. Padding rows up to L (multiple of 128) are killed with a -1e30 exp
bias so they contribute exactly 0, matching the reference's -inf masking.

Dataflow (all feature-major / "transposed", so no transposes are needed):
  xT [C, S]  (host-transposed)
  QT = WqT.T @ xqT       [256, 1024]  (feature-major)
  KT = WkT.T @ xkT       [256, L]
  V  = xkT.T @ WvT       [L, 256]    (token-major; lhsT for attnV)
  per head h: ST[kc] = KT_h[:,kc].T @ QT_h   [128, 1024] in PSUM  (contraction hd=32)
              PT[h][kc] = exp(scale*ST + maskbias)  -> SBUF bf16  (ACT, the bottleneck)
  attnV: 4-head col-tiled matmuls accumulate  O^T[32j:32j+32,:] += V_h[kc].T @ PT[h][kc]
  sums:  M=1 col-tiled ones-matmuls           S[32j,:] += 1.T @ PT[h][kc]
  softmax normalize: recip = 1/S (DVE), broadcast via E128 matmul (PE is the
  only partition-broadcast engine), xoutT = O^T * recip_bcast  (DVE)
  MLP: h1T = gelu(W1T.T @ xoutT + b1) (exact Gelu on ACT)
  final[s,:] = [h1T; xoutT].T @ [W2T; I]  -- residual fused via identity block
Biases: bq/bk folded into the projection PSUM->SBUF copies (per-partition
tensor_scalar add), b1 into the Gelu activation bias, bv added per-partition
after normalization (only if nonzero), b2 added on host (pure output offset).

Self-contained: hardcodes all shapes; compiled NEFF cached per L.
"""

import os
import sys

for _p in ("/opt/trn_rl_repo", "/root/.axon_site/_ro/trn_rl_repo"):
    if os.path.isdir(_p) and _p not in sys.path:
        sys.path.append(_p)

import ml_dtypes
import numpy as np

import concourse.bacc as bacc
import concourse.tile as tile
from concourse import mybir
from concourse.bass_utils import run_bass_kernel_spmd

B, S, C, H, HD = 4, 2048, 256, 8, 32
NCORES = 8
SQ = 1024                      # query rows per core
SCALE = float(HD) ** -0.5
F32 = mybir.dt.float32
BF16 = mybir.dt.bfloat16
AF = mybir.ActivationFunctionType
NEG = -1e30

_cache: dict = {}

F32R = mybir.dt.float32r



def _build(L: int, use_bv: bool, use_b: bool = True, stop_after: str | None = None):
    """Build the single-core program (SPMD across 8 cores)."""
    KC = L // 128
    nc = bacc.Bacc("TRN2", target_bir_lowering=False, debug=False, num_devices=NCORES)

    # ---- I/O ----
    d_xqT = nc.dram_tensor("xqT", [C, SQ], BF16, kind="ExternalInput")
    d_xkT = nc.dram_tensor("xkT", [C, L], BF16, kind="ExternalInput")
    d_wqT = nc.dram_tensor("wqT", [C, C], BF16, kind="ExternalInput")
    d_wkT = nc.dram_tensor("wkT", [C, C], BF16, kind="ExternalInput")
    d_wvT = nc.dram_tensor("wvT", [C, C], BF16, kind="ExternalInput")
    d_w1T = nc.dram_tensor("w1T", [C, C], F32R, kind="ExternalInput")
    d_w2TA = nc.dram_tensor("w2TA", [2 * C, C], F32R, kind="ExternalInput")
    d_mb = nc.dram_tensor("mb", [128, KC], F32, kind="ExternalInput")
    d_e128 = nc.dram_tensor("e128", [128, 128], F32R, kind="ExternalInput")
    d_bqk1 = nc.dram_tensor("bqk1", [128, 6], F32, kind="ExternalInput")  # bq|bk|b1 chunks
    d_bv = nc.dram_tensor("bv", [128, 2], F32, kind="ExternalInput")
    d_out = nc.dram_tensor("out", [SQ, C], F32, kind="ExternalOutput")

    kchunks = [(o, min(512, L - o)) for o in range(0, L, 512)]

    with tile.TileContext(nc) as tc:
        with tc.tile_pool(name="persist", bufs=1) as pp, \
             tc.tile_pool(name="pt", bufs=1) as ptp, \
             tc.tile_pool(name="work", bufs=4) as wp, \
             tc.tile_pool(name="ps_s", bufs=2, space="PSUM") as ps_s, \
             tc.tile_pool(name="ps_a", bufs=1, space="PSUM") as ps_a, \
             tc.tile_pool(name="ps_m", bufs=1, space="PSUM") as ps_m:

            # ---- load inputs ----
            xqT = [pp.tile([128, SQ], BF16, tag=f"xqT{m}", name=f"xqT{m}") for m in range(2)]
            xkT = [pp.tile([128, L], BF16, tag=f"xkT{m}", name=f"xkT{m}") for m in range(2)]
            wqT = [pp.tile([128, C], BF16, tag=f"wqT{m}", name=f"wqT{m}") for m in range(2)]
            wkT = [pp.tile([128, C], BF16, tag=f"wkT{m}", name=f"wkT{m}") for m in range(2)]
            wvT = [pp.tile([128, C], BF16, tag=f"wvT{m}", name=f"wvT{m}") for m in range(2)]
            w1T = [pp.tile([128, C], F32R, tag=f"w1T{m}", name=f"w1T{m}") for m in range(2)]
            w2TA = [pp.tile([128, C], F32R, tag=f"w2TA{m}", name=f"w2TA{m}") for m in range(4)]
            # first-exp critical path first: score weights, biases, mask
            # bias, then the x tiles; wv/w1/w2TA/e128/bv are consumed later
            bqk1 = pp.tile([128, 6], F32, tag="bqk1")
            mb = pp.tile([128, KC], F32, tag="mb")
            for m in range(2):
                nc.sync.dma_start(out=wqT[m], in_=d_wqT[m * 128:(m + 1) * 128, :])
                nc.sync.dma_start(out=wkT[m], in_=d_wkT[m * 128:(m + 1) * 128, :])
            if use_b:
                nc.sync.dma_start(out=bqk1, in_=d_bqk1[:, :])
            nc.sync.dma_start(out=mb, in_=d_mb[:, :])
            for m in range(2):
                nc.sync.dma_start(out=xkT[m], in_=d_xkT[m * 128:(m + 1) * 128, :])
                nc.sync.dma_start(out=xqT[m], in_=d_xqT[m * 128:(m + 1) * 128, :])
            for m in range(2):
                nc.sync.dma_start(out=wvT[m], in_=d_wvT[m * 128:(m + 1) * 128, :])
                nc.sync.dma_start(out=w1T[m], in_=d_w1T[m * 128:(m + 1) * 128, :])
            for m in range(4):
                nc.sync.dma_start(out=w2TA[m], in_=d_w2TA[m * 128:(m + 1) * 128, :])
            e128 = pp.tile([128, 128], F32R, tag="e128")
            nc.sync.dma_start(out=e128, in_=d_e128[:, :])
            bv = pp.tile([128, 2], F32, tag="bv")
            nc.sync.dma_start(out=bv, in_=d_bv[:, :])
            ones = pp.tile([128, 1], BF16, tag="ones")
            nc.vector.memset(ones, 1.0)

            # ---- projections (feature-major QT/KT, token-major V) ----
            QT = [pp.tile([128, SQ], F32R, tag=f"QT{m}", name=f"QT{m}") for m in range(2)]
            KT = [pp.tile([128, L], F32R, tag=f"KT{m}", name=f"KT{m}") for m in range(2)]
            V = [pp.tile([128, C], BF16, tag=f"V{sc}", name=f"V{sc}") for sc in range(KC)]

            alt = [0]
            def _ps():
                alt[0] ^= 1
                return ps_m.tile([128, 512], F32, tag=("proj" if alt[0] else "prb"),
                                 name="pp")
            def emit_qk_proj(m):
                for n in range(2):  # SQ/512
                    pq = _ps()
                    for kk in range(2):
                        nc.tensor.matmul(out=pq, lhsT=wqT[kk][:, m * 128:(m + 1) * 128],
                                         rhs=xqT[kk][:, n * 512:(n + 1) * 512],
                                         start=(kk == 0), stop=(kk == 1))
                    if use_b:
                        nc.vector.tensor_scalar_add(out=QT[m][:, n * 512:(n + 1) * 512],
                                                    in0=pq, scalar1=bqk1[:, m:m + 1])
                    else:
                        nc.vector.tensor_copy(out=QT[m][:, n * 512:(n + 1) * 512], in_=pq)
                for o, w in kchunks:
                    pk = _ps()
                    for kk in range(2):
                        nc.tensor.matmul(out=pk[:, :w], lhsT=wkT[kk][:, m * 128:(m + 1) * 128],
                                         rhs=xkT[kk][:, o:o + w],
                                         start=(kk == 0), stop=(kk == 1))
                    if use_b:
                        nc.vector.tensor_scalar_add(out=KT[m][:, o:o + w], in0=pk[:, :w],
                                                    scalar1=bqk1[:, 2 + m:3 + m])
                    else:
                        nc.vector.tensor_copy(out=KT[m][:, o:o + w], in_=pk[:, :w])

            emit_qk_proj(0)  # group-0 heads only; m=1 follows the first scores

            # ---- attention ----
            # Program order interleaves group 0's attnV/sums phase with group
            # 1's scores so the ACT engine (the bottleneck, running exps)
            # never starves while the PE drains a group's attnV accumulation.
            xoutT = [pp.tile([128, SQ], F32R, tag=f"xoutT{g}", name=f"xoutT{g}") for g in range(2)]
            PT = {}

            def emit_scores(g, kc):
                # adjacent heads hit different PE row-groups -> concurrent strips
                for j in range(4):
                    h = 4 * g + j
                    pss = ps_s.tile([128, SQ], F32, tag="scores", name="pss")
                    for qn in range(2):
                        nc.tensor.matmul(
                            out=pss[:, qn * 512:(qn + 1) * 512],
                            lhsT=KT[g][32 * j:32 * j + 32, kc * 128:(kc + 1) * 128],
                            rhs=QT[g][32 * j:32 * j + 32, qn * 512:(qn + 1) * 512],
                            start=True, stop=True,
                            tile_position=(32 * j, 0))
                    pt_t = ptp.tile([128, SQ], BF16, tag="pt", bufs=50,
                                    name=f"pt{h}_{kc}")
                    nc.scalar.activation(out=pt_t, in_=pss, func=AF.Exp,
                                         bias=mb[:, kc:kc + 1], scale=SCALE)
                    PT[h, kc] = pt_t

            def open_attn():
                po = ps_a.tile([128, 512], F32, tag="po", name="po")
                psum = ps_a.tile([128, 512], F32, tag="psum", name="psum")
                nc.vector.memset(psum, 1.0)
                return po, psum

            def emit_attn(g, qc, kc, po, psum):
                # sums (trivial ldweights) first, then the 4 attn col-strips
                # back-to-back -> concurrent on the PE array while the next
                # V ldweights pulls ahead through the reorder window
                for j in range(4):
                    h = 4 * g + j
                    nc.tensor.matmul(out=psum[32 * j:32 * j + 1, :],
                                     lhsT=ones[:, 0:1],
                                     rhs=PT[h, kc][:, qc * 512:(qc + 1) * 512],
                                     start=(kc == 0), stop=(kc == KC - 1),
                                     tile_position=(0, 32 * j),
                                     skip_group_check=(j > 0))
                for j in range(4):
                    h = 4 * g + j
                    nc.tensor.matmul(out=po[32 * j:32 * j + 32, :],
                                     lhsT=V[kc][:, h * 32:(h + 1) * 32],
                                     rhs=PT[h, kc][:, qc * 512:(qc + 1) * 512],
                                     start=(kc == 0), stop=(kc == KC - 1),
                                     tile_position=(0, 32 * j),
                                     skip_group_check=(j > 0))

            def emit_normalize(g, qc, po, psum):
                # evacuate po early (parallel with the reciprocal), then
                # multiply sbuf x psum directly - keeps the bcast psum read
                # legal (only one PSUM operand) and drops a copy from the
                # critical chain
                po_sb = wp.tile([128, 512], F32, tag="rb", name="po_sb")
                nc.vector.tensor_copy(out=po_sb, in_=po)
                rec = wp.tile([128, 512], F32, tag="rec", name="rec")
                nc.vector.reciprocal_approx_fast(out=rec, in_=psum)
                rec_r = wp.tile([128, 512], F32R, tag="rec_r", name="rec_r")
                nc.vector.tensor_copy(out=rec_r, in_=rec)
                prb = _ps()
                nc.tensor.matmul(out=prb, lhsT=e128, rhs=rec_r, start=True, stop=True)
                xo = xoutT[g][:, qc * 512:(qc + 1) * 512]
                nc.vector.tensor_mul(out=xo, in0=po_sb, in1=prb)
                if use_bv:
                    nc.vector.tensor_scalar_add(out=xo, in0=xo,
                                                scalar1=bv[:, g:g + 1])

            # ---- MLP + fused residual (emitted per q-half) ----
            h1T = [pp.tile([128, SQ], F32R, tag=f"h1T{j}", name=f"h1T{j}") for j in range(2)]

            def emit_mlp_half(n):
                for j in range(2):
                    ph = _ps()
                    for cc in range(2):
                        nc.tensor.matmul(out=ph, lhsT=w1T[cc][:, j * 128:(j + 1) * 128],
                                         rhs=xoutT[cc][:, n * 512:(n + 1) * 512],
                                         start=(cc == 0), stop=(cc == 1))
                    nc.scalar.activation(out=h1T[j][:, n * 512:(n + 1) * 512], in_=ph,
                                         func=AF.Gelu,
                                         bias=(bqk1[:, 4 + j:5 + j] if use_b else 0.0))
                for sc in range(4 * n, 4 * n + 4):
                    pf = _ps()
                    for cc in range(4):
                        lh = h1T[cc] if cc < 2 else xoutT[cc - 2]
                        nc.tensor.matmul(out=pf[:, :C],
                                         lhsT=lh[:, sc * 128:(sc + 1) * 128],
                                         rhs=w2TA[cc][:, :],
                                         start=(cc == 0), stop=(cc == 3))
                    ot = wp.tile([128, C], F32, tag="ot", name="ot")
                    # ACT is idle after the gelus; keep DVE free for normalize chains
                    nc.scalar.copy(out=ot, in_=pf[:, :C])
                    nc.sync.dma_start(out=d_out[sc * 128:(sc + 1) * 128, :], in_=ot)

            if stop_after != "proj":
                for kc in range(min(2, KC)):
                    emit_scores(0, kc)
                emit_qk_proj(1)  # heads 4-7, needed only from group 1 onward
                # V projections emitted after the scores pipeline is primed
                for sc in range(KC):
                    pv = _ps()
                    for kk in range(2):
                        nc.tensor.matmul(out=pv[:, :C],
                                         lhsT=xkT[kk][:, sc * 128:(sc + 1) * 128],
                                         rhs=wvT[kk][:, :], start=(kk == 0), stop=(kk == 1))
                    nc.vector.tensor_copy(out=V[sc], in_=pv[:, :C])
                for kc in range(2, KC):
                    emit_scores(0, kc)
                if stop_after != "scores":
                    # group 0 attnV interleaved with ~half of group 1's scores;
                    # the rest are emitted just-in-time inside group 1's qc=0
                    # attnV pass so the ACT exp pipeline never goes idle.
                    units = [(0, qc, kc) for qc in range(2) for kc in range(KC)]
                    sidx = 0
                    cur = None
                    for i, (g, qc, kc) in enumerate(units):
                        if i % 2 == 0 and sidx < KC:
                            emit_scores(1, sidx)
                            sidx += 1
                        if kc == 0:
                            cur = open_attn()
                        emit_attn(g, qc, kc, *cur)
                        if kc == KC - 1:
                            emit_normalize(g, qc, *cur)
                    # group 1 attnV; MLP half n follows normalize(1, n) so
                    # the n=0 MLP overlaps the qc=1 attnV accumulation
                    for qc in range(2):
                        cur = open_attn()
                        for kc in range(KC):
                            while sidx <= kc:
                                emit_scores(1, sidx)
                                sidx += 1
                            emit_attn(1, qc, kc, *cur)
                        while sidx < KC:
                            emit_scores(1, sidx)
                            sidx += 1
                        emit_normalize(1, qc, *cur)
                        if stop_after is None:
                            emit_mlp_half(qc)

            # (MLP emitted via emit_mlp_half above)

    nc.compile()
    return nc


def _prep_inputs(x, mask, Wq, bq, Wk, bk, Wv, bv, W1, b1, W2, b2):
    """Host-side sharding + layout prep. Returns (L, in_maps, use_bv)."""
    x = np.ascontiguousarray(x, dtype=np.float32)
    keeps = [np.flatnonzero(mask[b, :S] != 0) for b in range(B)]
    cnts = [len(k) for k in keeps]
    L = max(128, -(-max(cnts) // 128) * 128)
    KC = L // 128

    BF = ml_dtypes.bfloat16
    wqT = np.ascontiguousarray(np.asarray(Wq, np.float32).T.astype(BF))
    wkT = np.ascontiguousarray(np.asarray(Wk, np.float32).T.astype(BF))
    wvT = np.ascontiguousarray(np.asarray(Wv, np.float32).T.astype(BF))
    w1T = np.ascontiguousarray(W1.T, dtype=np.float32)
    w2TA = np.ascontiguousarray(
        np.vstack([W2.T.astype(np.float32), np.eye(C, dtype=np.float32)]))
    e128 = np.zeros((128, 128), dtype=np.float32)
    for m in range(128):
        e128[32 * (m // 32), m] = 1.0
    bqk1 = np.stack([
        bq[0:128], bq[128:256], bk[0:128], bk[128:256], b1[0:128], b1[128:256],
    ], axis=1).astype(np.float32)
    bv_t = np.stack([bv[0:128], bv[128:256]], axis=1).astype(np.float32)
    use_bv = bool(np.any(bv != 0))
    use_b = bool(np.any(bq != 0) or np.any(bk != 0) or np.any(b1 != 0))

    in_maps = []
    for core in range(NCORES):
        b, half = core // 2, core % 2
        xb = x[b]                                   # [S, C]
        xqT = np.ascontiguousarray(xb[half * SQ:(half + 1) * SQ].T.astype(BF))  # [C, SQ]
        xk = np.zeros((L, C), dtype=np.float32)
        xk[:cnts[b]] = xb[keeps[b]]
        xkT = np.ascontiguousarray(xk.T.astype(BF))  # [C, L]
        mb = np.full(L, NEG, dtype=np.float32)
        mb[:cnts[b]] = 0.0
        mb = np.ascontiguousarray(mb.reshape(KC, 128).T)  # [128, KC]
        in_maps.append({
            "xqT": xqT, "xkT": xkT, "wqT": wqT, "wkT": wkT, "wvT": wvT,
            "w1T": w1T, "w2TA": w2TA, "mb": mb, "e128": e128,
            "bqk1": bqk1, "bv": bv_t,
        })
    return L, in_maps, use_bv, use_b


def kernel(x, mask, Wq, bq, Wk, bk, Wv, bv, W1, b1, W2, b2):
    L, in_maps, use_bv, use_b = _prep_inputs(x, mask, Wq, bq, Wk, bk, Wv, bv, W1, b1, W2, b2)
    key = (L, use_bv, use_b)
    if key not in _cache:
        _cache[key] = _build(L, use_bv, use_b)
    nc = _cache[key]
    res = None
    last_exc = None
    for attempt in range(4):
        try:
            res = run_bass_kernel_spmd(nc, in_maps, core_ids=list(range(NCORES)),
                                       trace=False)
            break
        except Exception as e:  # transient device errors on first exec of a NEFF
            last_exc = e
            import time as _time
            import jax as _jax
            _time.sleep(2.0)
            try:
                _jax.clear_caches()
            except Exception:
                pass
    if res is None:
        raise last_exc
    out = np.empty((B, S, C), dtype=np.float32)
    for core in range(NCORES):
        b, half = core // 2, core % 2
        out[b, half * SQ:(half + 1) * SQ] = res.results[core]["out"]
    if np.any(b2 != 0):
        out += np.asarray(b2, dtype=np.float32)[None, None, :]
    # stash for test harness reuse (timing reruns)
    kernel.last = {"nc": nc, "in_maps": in_maps, "L": L}
    return out



# revision 8
# speedup vs baseline: 1.2302x; 1.2302x over previous
"""Trainium2 Bass kernel for nn_Block_softmoe (dense transformer block, B=4 S=2048 C=256 H=8).

Strategy (v2)
-------------
Sharding: 8 cores = (batch b, query-half). Each core computes the full block for
1024 query rows of one batch. K/V are computed per-core over that batch's keys
(2x redundant K/V projection; tiny at dim 256). No collectives.

Mask compaction: the key mask (Bernoulli 0/1) is applied on the host by
gathering only the kept key rows (~1024 of 2048), so L ~= 1024, KC = L/128.

Cost-model facts this kernel is built around:
  - matmul cost = out-free-size x cycles_per_row (bf16/f32r>=256: 1.0); the
    stationary (lhsT) load is free -> stream the SMALL operand.
  - only ACT has exp; DVE can fake it with the Schraudolph bit trick
    (int16(a*y+b) bitcast to bf16, saturation at -32768 gives -0.0 for
    masked keys), so the 64 exp tiles are SPLIT across ACT and DVE.

Dataflow per core (all sizes per core: SQ=1024 queries, L keys):
  QT = WqT.T @ xqT   [256, SQ] f32r (feature-major)   KT likewise [256, L]
  Vone[kc] [128, 8*33] bf16: per head h cols h*33..h*33+31 = V feats, +1 ones
  scores (h,kc): psum[128, SQ] = KT_h[kc].T @ QT_h   (PE, streams queries)
  P[h,kc] = exp(scale*scores + maskbias) -> bf16     (ACT exp | DVE bit trick)
  attnV (qc: 128-query chunk): po[128, 264] += P[h,kc][:,qc].T @ Vone[kc][h]
    -> ONE 33-wide stream gives attn.V AND the softmax denominator (PE)
  normalize: rec = 1/po[:,:,32] (DVE), xout[q,c] = po * rec (DVE, bcast AP)
  transpose xout -> feature-major xoutT via PE identity-transpose
  MLP: h1T = gelu(W1T.T @ xoutT + b1) (ACT), final = [h1T;xoutT].T @ [W2T;I]
  (residual fused via identity block), out copies on ACT -> DMA.

attnV for the first two query chunks is interleaved into the scores phase
(PSUM: 4 banks scores double-buffer + 2 banks po + 2 banks proj/mlp = 8).

Self-contained: hardcodes all shapes; compiled NEFF cached per L.
"""

import os
import sys

for _p in ("/opt/trn_rl_repo", "/root/.axon_site/_ro/trn_rl_repo"):
    if os.path.isdir(_p) and _p not in sys.path:
        sys.path.append(_p)

import ml_dtypes
import numpy as np

import concourse.bacc as bacc
import concourse.tile as tile
from concourse import mybir
from concourse.bass_utils import run_bass_kernel_spmd

B, S, C, H, HD = 4, 2048, 256, 8, 32
NCORES = 8
SQ = 1024                      # query rows per core
NQC = SQ // 128                # query chunks for attnV
SCALE = float(HD) ** -0.5
F32 = mybir.dt.float32
F32R = mybir.dt.float32r
BF16 = mybir.dt.bfloat16
I16 = mybir.dt.int16
AF = mybir.ActivationFunctionType
ALU = mybir.AluOpType
NEG = -1e30

# Schraudolph exp in bf16 bits: int16(A*y + B) viewed as bf16 ~= exp(y).
A_EXP = 2.0 ** 7 / np.log(2.0)
B_EXP = 127.0 * 2.0 ** 7 - 4.7
MB2_MASKED = -1e6              # saturates the int16 -> -32768 -> bf16 -0.0

# which exp units run on DVE (unit u = kc*8+h, pattern below repeats mod 8)
DVE_PAT = (1, 4, 6)

_cache: dict = {}


def _build(L: int, use_bv: bool, use_b: bool = True):
    """Build the single-core program (SPMD across 8 cores)."""
    KC = L // 128
    nc = bacc.Bacc("TRN2", target_bir_lowering=False, debug=False, num_devices=NCORES)

    # ---- I/O ----
    d_xqT = nc.dram_tensor("xqT", [C, SQ], BF16, kind="ExternalInput")
    d_xkT = nc.dram_tensor("xkT", [C, L], BF16, kind="ExternalInput")
    d_wqT = nc.dram_tensor("wqT", [C, C], BF16, kind="ExternalInput")
    d_wkT = nc.dram_tensor("wkT", [C, C], BF16, kind="ExternalInput")
    d_wvT = nc.dram_tensor("wvT", [C, C], BF16, kind="ExternalInput")
    d_w1T = nc.dram_tensor("w1T", [C, C], F32R, kind="ExternalInput")
    d_w2TA = nc.dram_tensor("w2TA", [2 * C, C], F32R, kind="ExternalInput")
    d_mb = nc.dram_tensor("mb", [128, KC], F32, kind="ExternalInput")
    d_mb2 = nc.dram_tensor("mb2", [128, KC], F32, kind="ExternalInput")
    d_ident = nc.dram_tensor("ident", [128, 128], F32R, kind="ExternalInput")
    d_bqk1 = nc.dram_tensor("bqk1", [128, 6], F32, kind="ExternalInput")  # bq|bk|b1
    d_bvrow = nc.dram_tensor("bvrow", [1, C], F32R, kind="ExternalInput")
    d_out = nc.dram_tensor("out", [SQ, C], F32, kind="ExternalOutput")

    with tile.TileContext(nc) as tc:
        with tc.tile_pool(name="persist", bufs=1) as pp, \
             tc.tile_pool(name="pt", bufs=1) as ptp, \
             tc.tile_pool(name="work", bufs=3) as wp, \
             tc.tile_pool(name="ps_s", bufs=2, space="PSUM") as ps_s, \
             tc.tile_pool(name="ps_po", bufs=2, space="PSUM") as ps_po, \
             tc.tile_pool(name="ps_m", bufs=1, space="PSUM") as ps_m:

            # ---- load inputs (first-scores critical path first) ----
            xqT = [pp.tile([128, SQ], BF16, tag=f"xqT{m}", name=f"xqT{m}") for m in range(2)]
            xkT = [pp.tile([128, L], BF16, tag=f"xkT{m}", name=f"xkT{m}") for m in range(2)]
            wqT = [pp.tile([128, C], BF16, tag=f"wqT{m}", name=f"wqT{m}") for m in range(2)]
            wkT = [pp.tile([128, C], BF16, tag=f"wkT{m}", name=f"wkT{m}") for m in range(2)]
            wvT = [pp.tile([128, C], BF16, tag=f"wvT{m}", name=f"wvT{m}") for m in range(2)]
            w1T = [pp.tile([128, C], F32R, tag=f"w1T{m}", name=f"w1T{m}") for m in range(2)]
            w2TA = [pp.tile([128, C], F32R, tag=f"w2TA{m}", name=f"w2TA{m}") for m in range(4)]
            bqk1 = pp.tile([128, 6], F32, tag="bqk1")
            mb = pp.tile([128, KC], F32, tag="mb")
            mb2 = pp.tile([128, KC], F32, tag="mb2")
            for m in range(2):
                nc.sync.dma_start(out=wqT[m], in_=d_wqT[m * 128:(m + 1) * 128, :])
                nc.sync.dma_start(out=wkT[m], in_=d_wkT[m * 128:(m + 1) * 128, :])
            if use_b:
                nc.sync.dma_start(out=bqk1, in_=d_bqk1[:, :])
            nc.sync.dma_start(out=mb, in_=d_mb[:, :])
            nc.sync.dma_start(out=mb2, in_=d_mb2[:, :])
            for m in range(2):
                nc.sync.dma_start(out=xqT[m], in_=d_xqT[m * 128:(m + 1) * 128, :])
                nc.sync.dma_start(out=xkT[m], in_=d_xkT[m * 128:(m + 1) * 128, :])
            for m in range(2):
                nc.sync.dma_start(out=wvT[m], in_=d_wvT[m * 128:(m + 1) * 128, :])
                nc.sync.dma_start(out=w1T[m], in_=d_w1T[m * 128:(m + 1) * 128, :])
            for m in range(4):
                nc.sync.dma_start(out=w2TA[m], in_=d_w2TA[m * 128:(m + 1) * 128, :])
            ident = pp.tile([128, 128], F32R, tag="ident")
            nc.sync.dma_start(out=ident, in_=d_ident[:, :])
            if use_bv:
                bvrow = pp.tile([1, C], F32R, tag="bvrow")
                onesr = pp.tile([1, 128], F32R, tag="onesr")
                nc.sync.dma_start(out=bvrow, in_=d_bvrow[:, :])
                nc.vector.memset(onesr, 1.0)

            # ---- persistent intermediates ----
            QT = [pp.tile([128, SQ], F32R, tag=f"QT{m}", name=f"QT{m}") for m in range(2)]
            KT = [pp.tile([128, L], F32R, tag=f"KT{m}", name=f"KT{m}") for m in range(2)]
            Vone = [pp.tile([128, H * 33], BF16, tag=f"Vone{sc}", name=f"Vone{sc}")
                    for sc in range(KC)]
            # feature-major attn output / mlp hidden: cols = cc*SQ + q
            xoutT = pp.tile([128, 2 * SQ], F32R, tag="xoutT", name="xoutT")
            h1T = [pp.tile([128, SQ], F32R, tag=f"h1T{j}", name=f"h1T{j}") for j in range(2)]
            PT = {}

            alt = [0]

            def _ps():
                alt[0] ^= 1
                return ps_m.tile([128, 512], F32, tag=("proj" if alt[0] else "prb"),
                                 name="pp")

            kchunks = [(o, min(512, L - o)) for o in range(0, L, 512)]

            def emit_qk_proj(m):
                for n in range(2):  # Q: SQ/512
                    pq = _ps()
                    for kk in range(2):
                        nc.tensor.matmul(out=pq, lhsT=wqT[kk][:, m * 128:(m + 1) * 128],
                                         rhs=xqT[kk][:, n * 512:(n + 1) * 512],
                                         start=(kk == 0), stop=(kk == 1))
                    if use_b:
                        nc.vector.tensor_scalar_add(out=QT[m][:, n * 512:(n + 1) * 512],
                                                    in0=pq, scalar1=bqk1[:, m:m + 1])
                    else:
                        nc.scalar.copy(out=QT[m][:, n * 512:(n + 1) * 512], in_=pq)
                for o, w in kchunks:
                    pk = _ps()
                    for kk in range(2):
                        nc.tensor.matmul(out=pk[:, :w], lhsT=wkT[kk][:, m * 128:(m + 1) * 128],
                                         rhs=xkT[kk][:, o:o + w],
                                         start=(kk == 0), stop=(kk == 1))
                    if use_b:
                        nc.vector.tensor_scalar_add(out=KT[m][:, o:o + w], in0=pk[:, :w],
                                                    scalar1=bqk1[:, 2 + m:3 + m])
                    else:
                        nc.vector.tensor_copy(out=KT[m][:, o:o + w], in_=pk[:, :w])

            def emit_v_proj(sc):
                pv = _ps()
                for kk in range(2):
                    nc.tensor.matmul(out=pv[:, :C],
                                     lhsT=xkT[kk][:, sc * 128:(sc + 1) * 128],
                                     rhs=wvT[kk][:, :], start=(kk == 0),
                                     stop=(kk == 1) and not use_bv)
                if use_bv:
                    nc.tensor.matmul(out=pv[:, :C], lhsT=onesr[0:1, :],
                                     rhs=bvrow[0:1, :], start=False, stop=True)
                vr = Vone[sc][:, :].rearrange("p (h w) -> p h w", h=H)
                nc.vector.tensor_copy(out=vr[:, :, 0:32],
                                      in_=pv[:, :C].rearrange("p (h w) -> p h w", h=H))
                nc.vector.memset(vr[:, :, 32:33], 1.0)

            uidx = [0]

            def emit_score_exp(h, kc):
                g, j = h // 4, h % 4
                pss = ps_s.tile([128, SQ], F32, tag="scores", name="pss")
                for qn in range(2):
                    nc.tensor.matmul(
                        out=pss[:, qn * 512:(qn + 1) * 512],
                        lhsT=KT[g][32 * j:32 * j + 32, kc * 128:(kc + 1) * 128],
                        rhs=QT[g][32 * j:32 * j + 32, qn * 512:(qn + 1) * 512],
                        start=True, stop=True,
                        tile_position=(32 * j, 0))
                pt_t = ptp.tile([128, SQ], BF16, tag="pt", bufs=8 * KC,
                                name=f"pt{h}_{kc}")
                if (uidx[0] % 8) in DVE_PAT:
                    nc.vector.tensor_scalar(out=pt_t.bitcast(I16), in0=pss,
                                            scalar1=float(SCALE * A_EXP),
                                            scalar2=mb2[:, kc:kc + 1],
                                            op0=ALU.mult, op1=ALU.add)
                else:
                    nc.scalar.activation(out=pt_t, in_=pss, func=AF.Exp,
                                         bias=mb[:, kc:kc + 1], scale=SCALE)
                uidx[0] += 1
                PT[h, kc] = pt_t

            po_of = {}

            def emit_attn(qc, kc):
                if kc == 0:
                    po_of[qc] = ps_po.tile([128, H * 33], F32, tag="po", name=f"po{qc}")
                po = po_of[qc]
                for h in range(H):
                    # start=True zeroes the WHOLE psum bank (pending-zero
                    # region), so only the very first write may set it; all
                    # later head/kc writes accumulate onto pending-zero.
                    nc.tensor.matmul(
                        out=po[:, h * 33:(h + 1) * 33],
                        lhsT=PT[h, kc][:, qc * 128:(qc + 1) * 128],
                        rhs=Vone[kc][:, h * 33:(h + 1) * 33],
                        start=(kc == 0 and h == 0), stop=(kc == KC - 1),
                        skip_group_check=(h > 0))

            def emit_norm_transpose(qc):
                po = po_of[qc][:, :].rearrange("p (h w) -> p h w", h=H)
                rec = wp.tile([128, H, 1], F32, tag="rec", name="rec")
                nc.vector.reciprocal(out=rec, in_=po[:, :, 32:33])
                xo = wp.tile([128, C], F32R, tag="xo", name="xo")
                nc.vector.tensor_mul(out=xo[:, :].rearrange("p (h w) -> p h w", h=H),
                                     in0=po[:, :, 0:32],
                                     in1=rec[:, :, :].broadcast_to((128, H, 32)))
                ptr = _ps()
                for cc in range(2):
                    nc.tensor.transpose(out=ptr[:, cc * 128:(cc + 1) * 128].bitcast(F32R),
                                        in_=xo[:, cc * 128:(cc + 1) * 128],
                                        identity=ident)
                xoT = xoutT[:, :].rearrange("p (c q) -> p c q", c=2)
                nc.vector.tensor_copy(
                    out=xoT[:, :, qc * 128:(qc + 1) * 128],
                    in_=ptr[:, :C].rearrange("p (c q) -> p c q", c=2).bitcast(F32R))

            def emit_mlp_half(n):
                for j in range(2):
                    ph = _ps()
                    for cc in range(2):
                        nc.tensor.matmul(out=ph, lhsT=w1T[cc][:, j * 128:(j + 1) * 128],
                                         rhs=xoutT[:, cc * SQ + n * 512:cc * SQ + (n + 1) * 512],
                                         start=(cc == 0), stop=(cc == 1))
                    nc.scalar.activation(out=h1T[j][:, n * 512:(n + 1) * 512], in_=ph,
                                         func=AF.Gelu,
                                         bias=(bqk1[:, 4 + j:5 + j] if use_b else 0.0))
                for sc in range(4 * n, 4 * n + 4):
                    pf = _ps()
                    for cc in range(4):
                        lh = (h1T[cc][:, sc * 128:(sc + 1) * 128] if cc < 2 else
                              xoutT[:, (cc - 2) * SQ + sc * 128:(cc - 2) * SQ + (sc + 1) * 128])
                        nc.tensor.matmul(out=pf[:, :C], lhsT=lh, rhs=w2TA[cc][:, :],
                                         start=(cc == 0), stop=(cc == 3))
                    ot = wp.tile([128, C], F32, tag="ot", name="ot")
                    nc.scalar.copy(out=ot, in_=pf[:, :C])
                    nc.sync.dma_start(out=d_out[sc * 128:(sc + 1) * 128, :], in_=ot)

            # ---- emission schedule ----
            emit_qk_proj(0)
            for h in range(4):
                emit_score_exp(h, 0)
            emit_qk_proj(1)
            for h in range(4, 8):
                emit_score_exp(h, 0)
            for sc in range(min(4, KC)):
                emit_v_proj(sc)
            if KC > 1:
                for h in range(H):
                    emit_score_exp(h, 1)
            for sc in range(4, KC):
                emit_v_proj(sc)
            # scores kc>=2, with attnV for qc 0/1 interleaved 3 batches behind
            for kc in range(2, KC):
                for h in range(H):
                    emit_score_exp(h, kc)
                akc = kc - 3
                if 0 <= akc <= KC - 4:
                    emit_attn(0, akc)
                    emit_attn(1, akc)
            # tail: finish qc0/qc1, then remaining chunks
            for akc in range(max(0, KC - 3), KC):
                emit_attn(0, akc)
                emit_attn(1, akc)
            emit_norm_transpose(0)
            emit_norm_transpose(1)
            for qc in range(2, NQC):
                for kc in range(KC):
                    emit_attn(qc, kc)
                emit_norm_transpose(qc)
                if qc == 3:
                    emit_mlp_half(0)
            emit_mlp_half(1)

    nc.compile()
    return nc


def _prep_inputs(x, mask, Wq, bq, Wk, bk, Wv, bv, W1, b1, W2, b2):
    """Host-side sharding + layout prep. Returns (L, in_maps, use_bv, use_b)."""
    x = np.ascontiguousarray(x, dtype=np.float32)
    keeps = [np.flatnonzero(mask[b, :S] != 0) for b in range(B)]
    cnts = [len(k) for k in keeps]
    L = max(128, -(-max(cnts) // 128) * 128)
    KC = L // 128

    BF = ml_dtypes.bfloat16
    wqT = np.ascontiguousarray(np.asarray(Wq, np.float32).T.astype(BF))
    wkT = np.ascontiguousarray(np.asarray(Wk, np.float32).T.astype(BF))
    wvT = np.ascontiguousarray(np.asarray(Wv, np.float32).T.astype(BF))
    w1T = np.ascontiguousarray(W1.T, dtype=np.float32)
    w2TA = np.ascontiguousarray(
        np.vstack([W2.T.astype(np.float32), np.eye(C, dtype=np.float32)]))
    ident = np.eye(128, dtype=np.float32)
    bqk1 = np.stack([
        bq[0:128], bq[128:256], bk[0:128], bk[128:256], b1[0:128], b1[128:256],
    ], axis=1).astype(np.float32)
    bvrow = np.asarray(bv, np.float32).reshape(1, C)
    use_bv = bool(np.any(bv != 0))
    use_b = bool(np.any(bq != 0) or np.any(bk != 0) or np.any(b1 != 0))

    in_maps = []
    for core in range(NCORES):
        b, half = core // 2, core % 2
        xb = x[b]                                   # [S, C]
        xqT = np.ascontiguousarray(xb[half * SQ:(half + 1) * SQ].T.astype(BF))  # [C, SQ]
        xk = np.zeros((L, C), dtype=np.float32)
        xk[:cnts[b]] = xb[keeps[b]]
        xkT = np.ascontiguousarray(xk.T.astype(BF))  # [C, L]
        mb1d = np.full(L, NEG, dtype=np.float32)
        mb1d[:cnts[b]] = 0.0
        mb = np.ascontiguousarray(mb1d.reshape(KC, 128).T)  # [128, KC]
        mb2_1d = np.full(L, MB2_MASKED, dtype=np.float32)
        mb2_1d[:cnts[b]] = B_EXP
        mb2 = np.ascontiguousarray(mb2_1d.reshape(KC, 128).T)  # [128, KC]
        in_maps.append({
            "xqT": xqT, "xkT": xkT, "wqT": wqT, "wkT": wkT, "wvT": wvT,
            "w1T": w1T, "w2TA": w2TA, "mb": mb, "mb2": mb2, "ident": ident,
            "bqk1": bqk1, "bvrow": bvrow,
        })
    return L, in_maps, use_bv, use_b


def kernel(x, mask, Wq, bq, Wk, bk, Wv, bv, W1, b1, W2, b2):
    L, in_maps, use_bv, use_b = _prep_inputs(x, mask, Wq, bq, Wk, bk, Wv, bv, W1, b1, W2, b2)
    key = (L, use_bv, use_b)
    if key not in _cache:
        _cache[key] = _build(L, use_bv, use_b)
    nc = _cache[key]
    res = None
    last_exc = None
    for attempt in range(4):
        try:
            res = run_bass_kernel_spmd(nc, in_maps, core_ids=list(range(NCORES)),
                                       trace=False)
            break
        except Exception as e:  # transient device errors on first exec of a NEFF
            last_exc = e
            import time as _time
            import jax as _jax
            _time.sleep(2.0)
            try:
                _jax.clear_caches()
            except Exception:
                pass
    if res is None:
        raise last_exc
    out = np.empty((B, S, C), dtype=np.float32)
    for core in range(NCORES):
        b, half = core // 2, core % 2
        out[b, half * SQ:(half + 1) * SQ] = res.results[core]["out"]
    if np.any(b2 != 0):
        out += np.asarray(b2, dtype=np.float32)[None, None, :]
    # stash for test harness reuse (timing reruns)
    kernel.last = {"nc": nc, "in_maps": in_maps, "L": L}
    return out


# revision 9
# speedup vs baseline: 1.3034x; 1.0595x over previous
"""Trainium2 Bass kernel for nn_Block_softmoe (dense transformer block, B=4 S=2048 C=256 H=8).

Strategy (v3)
-------------
Sharding: 8 cores = (batch b, query-half). Each core computes the full block for
1024 query rows of one batch. K/V are computed per-core over that batch's keys
(2x redundant K/V projection; tiny at dim 256). No collectives.

Mask compaction: the key mask (Bernoulli 0/1) is applied on the host by
gathering only the kept key rows (~1024 of 2048), so L ~= 1024, KC = L/128.

Cost-model facts this kernel is built around:
  - matmul cost = out-free-size x cycles_per_row (bf16/f32r>=256: 1.0); the
    stationary (lhsT) load is free -> stream the SMALL operand.
  - only ACT has exp; DVE fakes it with the Schraudolph bit trick
    (int16(a*y+b) bitcast to bf16; int16 saturation gives -0.0 for masked
    keys), so the 64 exp tiles are SPLIT across ACT and DVE (Bresenham
    interleave, ACT-heavy since DVE carries more copy work).
  - every HWDGE dma_start serializes ~625ns on the single HWDGE queue ->
    consolidate input DMAs and push non-critical loads + half the output
    stores through the Pool engine's SWDGE path (Pool is otherwise idle;
    GPSIMD cannot touch PSUM so it only gets SBUF/DRAM work).

Dataflow per core (SQ=1024 queries, L keys):
  QT = WqT.T @ xqT   [256, SQ] f32r (feature-major)   KT likewise [256, L]
  Vone[kc] [128, 8*33] bf16: per head h cols h*33..h*33+31 = V feats, +1 ones
  scores (h,kc): psum[128, SQ] = KT_h[kc].T @ QT_h   (PE, streams queries)
  P[h,kc] = exp(scale*scores + maskbias) -> bf16     (ACT exp | DVE bit trick)
  attnV (qc: 128-query chunk): po[128, 264] += P[h,kc][:,qc].T @ Vone[kc][h]
    -> ONE 33-wide stream gives attn.V AND the softmax denominator (PE).
    NOTE start=True pending-zeroes the whole PSUM bank, so only the first
    write into the po bank sets it.
  normalize: rec = 1/po[:,:,32] (DVE), xout[q,c] = po * rec (DVE, bcast AP)
  transpose xout -> feature-major xoutT via PE identity-transpose
  MLP in 256-col quarter strips (overlaps the attnV tail):
  h1T = gelu(W1T.T @ xoutT + b1) (ACT), final = [h1T;xoutT].T @ [W2T;I]
  (residual fused via identity block), out copies ACT -> DMA (SP/Pool split).

attnV for the first two query chunks is interleaved into the scores phase
(PSUM banks: 4 scores double-buffer + 2 po + 2 proj/mlp = 8).

Self-contained: hardcodes all shapes; compiled NEFF cached per L.
"""

import os
import sys

for _p in ("/opt/trn_rl_repo", "/root/.axon_site/_ro/trn_rl_repo"):
    if os.path.isdir(_p) and _p not in sys.path:
        sys.path.append(_p)

import ml_dtypes
import numpy as np

import concourse.bacc as bacc
import concourse.tile as tile
from concourse import mybir
from concourse.bass_utils import run_bass_kernel_spmd

B, S, C, H, HD = 4, 2048, 256, 8, 32
NCORES = 8
SQ = 1024                      # query rows per core
NQC = SQ // 128                # query chunks for attnV
SCALE = float(HD) ** -0.5
F32 = mybir.dt.float32
F32R = mybir.dt.float32r
BF16 = mybir.dt.bfloat16
I16 = mybir.dt.int16
AF = mybir.ActivationFunctionType
ALU = mybir.AluOpType
NEG = -1e30

# Schraudolph exp in bf16 bits: int16(A*y + B) viewed as bf16 ~= exp(y).
A_EXP = 2.0 ** 7 / np.log(2.0)
B_EXP = 127.0 * 2.0 ** 7 - 4.7
MB2_MASKED = -1e6              # saturates the int16 -> -32768 -> bf16 -0.0

N_DVE_EXP = 29                 # of the 64 exp units, how many go to DVE

_cache: dict = {}


def _build(L: int, use_bv: bool, use_b: bool = True):
    """Build the single-core program (SPMD across 8 cores)."""
    KC = L // 128
    nc = bacc.Bacc("TRN2", target_bir_lowering=False, debug=False, num_devices=NCORES)

    # ---- I/O ----
    d_xqT = nc.dram_tensor("xqT", [C, SQ], BF16, kind="ExternalInput")
    d_xkT = nc.dram_tensor("xkT", [C, L], BF16, kind="ExternalInput")
    d_wqT = nc.dram_tensor("wqT", [C, C], BF16, kind="ExternalInput")
    d_wkT = nc.dram_tensor("wkT", [C, C], BF16, kind="ExternalInput")
    d_wvT = nc.dram_tensor("wvT", [C, C], BF16, kind="ExternalInput")
    d_w1T = nc.dram_tensor("w1T", [C, C], F32R, kind="ExternalInput")
    d_w2TA = nc.dram_tensor("w2TA", [2 * C, C], F32R, kind="ExternalInput")
    d_mbb = nc.dram_tensor("mbb", [128, 2 * KC], F32, kind="ExternalInput")  # mb|mb2
    d_ident = nc.dram_tensor("ident", [128, 128], F32R, kind="ExternalInput")
    d_bqk1 = nc.dram_tensor("bqk1", [128, 6], F32, kind="ExternalInput")  # bq|bk|b1
    d_bvrow = nc.dram_tensor("bvrow", [1, C], F32R, kind="ExternalInput")
    d_out = nc.dram_tensor("out", [SQ, C], F32, kind="ExternalOutput")

    # DVE/ACT exp assignment: Bresenham spread of N_DVE_EXP Ds over 8*KC units
    nu = 8 * KC
    nd = min(N_DVE_EXP, nu)
    dve_units = {u for u in range(nu)
                 if (u * nd) // nu != ((u + 1) * nd) // nu}

    with tile.TileContext(nc) as tc:
        with tc.tile_pool(name="persist", bufs=1) as pp, \
             tc.tile_pool(name="pt", bufs=1) as ptp, \
             tc.tile_pool(name="work", bufs=3) as wp, \
             tc.tile_pool(name="ps_s", bufs=2, space="PSUM") as ps_s, \
             tc.tile_pool(name="ps_po", bufs=2, space="PSUM") as ps_po, \
             tc.tile_pool(name="ps_m", bufs=1, space="PSUM") as ps_m:

            # ---- consolidated tiles (chunk-major columns) ----
            xqT = pp.tile([128, 2 * SQ], BF16, tag="xqT", name="xqT")    # cols kk*SQ+q
            xkT = pp.tile([128, 2 * L], BF16, tag="xkT", name="xkT")     # cols kk*L+t
            wqT = pp.tile([128, 2 * C], BF16, tag="wqT", name="wqT")     # cols kk*C+f
            wkT = pp.tile([128, 2 * C], BF16, tag="wkT", name="wkT")
            wvT = pp.tile([128, 2 * C], BF16, tag="wvT", name="wvT")
            w1T = pp.tile([128, 2 * C], F32R, tag="w1T", name="w1T")
            w2TA = pp.tile([128, 4 * C], F32R, tag="w2TA", name="w2TA")  # cols cc*C+f
            mbb = pp.tile([128, 2 * KC], F32, tag="mbb", name="mbb")
            ident = pp.tile([128, 128], F32R, tag="ident", name="ident")

            def _chunks(dram, tile_t, nch, w, dt_):
                # one DMA: DRAM [nch*128, w] -> SBUF [128, nch*w] chunk-major
                nc_ = dram[:, :].rearrange("(c p) w -> p c w", c=nch)
                nc.sync.dma_start(out=tile_t[:, :].rearrange("p (c w) -> p c w", c=nch),
                                  in_=nc_)

            # critical path on HWDGE: wq, xq, wk, xk
            _chunks(d_wqT, wqT, 2, C, BF16)
            _chunks(d_xqT, xqT, 2, SQ, BF16)
            _chunks(d_wkT, wkT, 2, C, BF16)
            _chunks(d_xkT, xkT, 2, L, BF16)
            # bulk loads via Pool SWDGE (off the HWDGE queue)
            nc.gpsimd.dma_start(out=mbb, in_=d_mbb[:, :])
            nc.gpsimd.dma_start(out=wvT[:, :].rearrange("p (c w) -> p c w", c=2),
                                in_=d_wvT[:, :].rearrange("(c p) w -> p c w", c=2))
            if use_b:
                bqk1 = pp.tile([128, 6], F32, tag="bqk1", name="bqk1")
                nc.gpsimd.dma_start(out=bqk1, in_=d_bqk1[:, :])
            nc.gpsimd.dma_start(out=ident, in_=d_ident[:, :])
            nc.gpsimd.dma_start(out=w1T[:, :].rearrange("p (c w) -> p c w", c=2),
                                in_=d_w1T[:, :].rearrange("(c p) w -> p c w", c=2))
            nc.gpsimd.dma_start(out=w2TA[:, :].rearrange("p (c w) -> p c w", c=4),
                                in_=d_w2TA[:, :].rearrange("(c p) w -> p c w", c=4))
            if use_bv:
                bvrow = pp.tile([1, C], F32R, tag="bvrow", name="bvrow")
                onesr = pp.tile([1, 128], F32R, tag="onesr", name="onesr")
                nc.gpsimd.dma_start(out=bvrow, in_=d_bvrow[:, :])
                nc.vector.memset(onesr, 1.0)

            mb = mbb[:, 0:KC]
            mb2 = mbb[:, KC:2 * KC]

            # ---- persistent intermediates ----
            QT = [pp.tile([128, SQ], F32R, tag=f"QT{m}", name=f"QT{m}") for m in range(2)]
            KT = [pp.tile([128, L], F32R, tag=f"KT{m}", name=f"KT{m}") for m in range(2)]
            Vone = [pp.tile([128, H * 33], BF16, tag=f"Vone{sc}", name=f"Vone{sc}")
                    for sc in range(KC)]
            # feature-major attn output: cols = cc*SQ + q
            xoutT = pp.tile([128, 2 * SQ], F32R, tag="xoutT", name="xoutT")
            h1T = [pp.tile([128, SQ], F32R, tag=f"h1T{j}", name=f"h1T{j}") for j in range(2)]
            PT = {}

            alt = [0]

            def _ps():
                alt[0] ^= 1
                return ps_m.tile([128, 512], F32, tag=("proj" if alt[0] else "prb"),
                                 name="pp")

            kchunks = [(o, min(512, L - o)) for o in range(0, L, 512)]

            def emit_qk_proj(m):
                for n in range(2):  # Q: SQ/512
                    pq = _ps()
                    for kk in range(2):
                        nc.tensor.matmul(out=pq, lhsT=wqT[:, kk * C + m * 128:kk * C + (m + 1) * 128],
                                         rhs=xqT[:, kk * SQ + n * 512:kk * SQ + (n + 1) * 512],
                                         start=(kk == 0), stop=(kk == 1))
                    if use_b:
                        nc.vector.tensor_scalar_add(out=QT[m][:, n * 512:(n + 1) * 512],
                                                    in0=pq, scalar1=bqk1[:, m:m + 1])
                    else:
                        nc.scalar.copy(out=QT[m][:, n * 512:(n + 1) * 512], in_=pq)
                for o, w in kchunks:
                    pk = _ps()
                    for kk in range(2):
                        nc.tensor.matmul(out=pk[:, :w], lhsT=wkT[:, kk * C + m * 128:kk * C + (m + 1) * 128],
                                         rhs=xkT[:, kk * L + o:kk * L + o + w],
                                         start=(kk == 0), stop=(kk == 1))
                    if use_b:
                        nc.vector.tensor_scalar_add(out=KT[m][:, o:o + w], in0=pk[:, :w],
                                                    scalar1=bqk1[:, 2 + m:3 + m])
                    else:
                        nc.vector.tensor_copy(out=KT[m][:, o:o + w], in_=pk[:, :w])

            def emit_v_proj(sc):
                pv = _ps()
                for kk in range(2):
                    nc.tensor.matmul(out=pv[:, :C],
                                     lhsT=xkT[:, kk * L + sc * 128:kk * L + (sc + 1) * 128],
                                     rhs=wvT[:, kk * C:(kk + 1) * C], start=(kk == 0),
                                     stop=(kk == 1) and not use_bv)
                if use_bv:
                    nc.tensor.matmul(out=pv[:, :C], lhsT=onesr[0:1, :],
                                     rhs=bvrow[0:1, :], start=False, stop=True)
                vr = Vone[sc][:, :].rearrange("p (h w) -> p h w", h=H)
                nc.vector.tensor_copy(out=vr[:, :, 0:32],
                                      in_=pv[:, :C].rearrange("p (h w) -> p h w", h=H))
                nc.gpsimd.memset(vr[:, :, 32:33], 1.0)

            uidx = [0]

            def emit_score_exp(h, kc):
                g, j = h // 4, h % 4
                pss = ps_s.tile([128, SQ], F32, tag="scores", name="pss")
                for qn in range(2):
                    nc.tensor.matmul(
                        out=pss[:, qn * 512:(qn + 1) * 512],
                        lhsT=KT[g][32 * j:32 * j + 32, kc * 128:(kc + 1) * 128],
                        rhs=QT[g][32 * j:32 * j + 32, qn * 512:(qn + 1) * 512],
                        start=True, stop=True,
                        tile_position=(32 * j, 0))
                pt_t = ptp.tile([128, SQ], BF16, tag="pt", bufs=8 * KC,
                                name=f"pt{h}_{kc}")
                if uidx[0] in dve_units:
                    nc.vector.tensor_scalar(out=pt_t.bitcast(I16), in0=pss,
                                            scalar1=float(SCALE * A_EXP),
                                            scalar2=mb2[:, kc:kc + 1],
                                            op0=ALU.mult, op1=ALU.add)
                else:
                    nc.scalar.activation(out=pt_t, in_=pss, func=AF.Exp,
                                         bias=mb[:, kc:kc + 1], scale=SCALE)
                uidx[0] += 1
                PT[h, kc] = pt_t

            po_of = {}

            def emit_attn(qc, kc):
                if kc == 0:
                    po_of[qc] = ps_po.tile([128, H * 33], F32, tag="po", name=f"po{qc}")
                po = po_of[qc]
                for h in range(H):
                    # start=True pending-zeroes the WHOLE psum bank, so only
                    # the very first write into the bank may set it.
                    nc.tensor.matmul(
                        out=po[:, h * 33:(h + 1) * 33],
                        lhsT=PT[h, kc][:, qc * 128:(qc + 1) * 128],
                        rhs=Vone[kc][:, h * 33:(h + 1) * 33],
                        start=(kc == 0 and h == 0), stop=(kc == KC - 1),
                        skip_group_check=(h > 0))

            def emit_norm_transpose(qc):
                po = po_of[qc][:, :].rearrange("p (h w) -> p h w", h=H)
                rec = wp.tile([128, H, 1], F32, tag="rec", name="rec")
                nc.vector.reciprocal(out=rec, in_=po[:, :, 32:33])
                xo = wp.tile([128, C], F32R, tag="xo", name="xo")
                nc.vector.tensor_mul(out=xo[:, :].rearrange("p (h w) -> p h w", h=H),
                                     in0=po[:, :, 0:32],
                                     in1=rec[:, :, :].broadcast_to((128, H, 32)))
                ptr = _ps()
                for cc in range(2):
                    nc.tensor.transpose(out=ptr[:, cc * 128:(cc + 1) * 128].bitcast(F32R),
                                        in_=xo[:, cc * 128:(cc + 1) * 128],
                                        identity=ident)
                xoT = xoutT[:, :].rearrange("p (c q) -> p c q", c=2)
                nc.vector.tensor_copy(
                    out=xoT[:, :, qc * 128:(qc + 1) * 128],
                    in_=ptr[:, :C].rearrange("p (c q) -> p c q", c=2).bitcast(F32R))

            def emit_mlp_quarter(n):
                # 256-col strip: queries n*256..(n+1)*256 (query chunks 2n, 2n+1)
                for j in range(2):
                    ph = _ps()
                    for cc in range(2):
                        nc.tensor.matmul(out=ph[:, 0:256], lhsT=w1T[:, cc * C + j * 128:cc * C + (j + 1) * 128],
                                         rhs=xoutT[:, cc * SQ + n * 256:cc * SQ + (n + 1) * 256],
                                         start=(cc == 0), stop=(cc == 1))
                    nc.scalar.activation(out=h1T[j][:, n * 256:(n + 1) * 256], in_=ph[:, 0:256],
                                         func=AF.Gelu,
                                         bias=(bqk1[:, 4 + j:5 + j] if use_b else 0.0))
                for sc in range(2 * n, 2 * n + 2):
                    pf = _ps()
                    for cc in range(4):
                        lh = (h1T[cc][:, sc * 128:(sc + 1) * 128] if cc < 2 else
                              xoutT[:, (cc - 2) * SQ + sc * 128:(cc - 2) * SQ + (sc + 1) * 128])
                        nc.tensor.matmul(out=pf[:, :C], lhsT=lh, rhs=w2TA[:, cc * C:(cc + 1) * C],
                                         start=(cc == 0), stop=(cc == 3))
                    ot = wp.tile([128, C], F32, tag="ot", name="ot")
                    nc.scalar.copy(out=ot, in_=pf[:, :C])
                    if sc % 2 == 0:
                        nc.sync.dma_start(out=d_out[sc * 128:(sc + 1) * 128, :], in_=ot)
                    else:
                        nc.gpsimd.dma_start(out=d_out[sc * 128:(sc + 1) * 128, :], in_=ot)

            # ---- emission schedule ----
            emit_qk_proj(0)
            for h in range(4):
                emit_score_exp(h, 0)
            emit_qk_proj(1)
            for h in range(4, 8):
                emit_score_exp(h, 0)
            for sc in range(min(4, KC)):
                emit_v_proj(sc)
            if KC > 1:
                for h in range(H):
                    emit_score_exp(h, 1)
            for sc in range(4, KC):
                emit_v_proj(sc)
            # scores kc>=2, with attnV for qc 0/1 interleaved 3 batches behind
            for kc in range(2, KC):
                for h in range(H):
                    emit_score_exp(h, kc)
                akc = kc - 3
                if 0 <= akc <= KC - 4:
                    emit_attn(0, akc)
                    emit_attn(1, akc)
            # tail: finish qc0/qc1, then remaining chunks; MLP quarter n
            # follows query-chunk 2n+1
            for akc in range(max(0, KC - 3), KC):
                emit_attn(0, akc)
                emit_attn(1, akc)
            emit_norm_transpose(0)
            emit_norm_transpose(1)
            emit_mlp_quarter(0)
            for qc in range(2, NQC):
                for kc in range(KC):
                    emit_attn(qc, kc)
                emit_norm_transpose(qc)
                if qc % 2 == 1:
                    emit_mlp_quarter(qc // 2)

    nc.compile()
    return nc


def _prep_inputs(x, mask, Wq, bq, Wk, bk, Wv, bv, W1, b1, W2, b2):
    """Host-side sharding + layout prep. Returns (L, in_maps, use_bv, use_b)."""
    x = np.ascontiguousarray(x, dtype=np.float32)
    keeps = [np.flatnonzero(mask[b, :S] != 0) for b in range(B)]
    cnts = [len(k) for k in keeps]
    L = max(128, -(-max(cnts) // 128) * 128)
    KC = L // 128

    BF = ml_dtypes.bfloat16
    wqT = np.ascontiguousarray(np.asarray(Wq, np.float32).T.astype(BF))
    wkT = np.ascontiguousarray(np.asarray(Wk, np.float32).T.astype(BF))
    wvT = np.ascontiguousarray(np.asarray(Wv, np.float32).T.astype(BF))
    w1T = np.ascontiguousarray(W1.T, dtype=np.float32)
    w2TA = np.ascontiguousarray(
        np.vstack([W2.T.astype(np.float32), np.eye(C, dtype=np.float32)]))
    ident = np.eye(128, dtype=np.float32)
    bqk1 = np.stack([
        bq[0:128], bq[128:256], bk[0:128], bk[128:256], b1[0:128], b1[128:256],
    ], axis=1).astype(np.float32)
    bvrow = np.asarray(bv, np.float32).reshape(1, C)
    use_bv = bool(np.any(bv != 0))
    use_b = bool(np.any(bq != 0) or np.any(bk != 0) or np.any(b1 != 0))

    in_maps = []
    for core in range(NCORES):
        b, half = core // 2, core % 2
        xb = x[b]                                   # [S, C]
        xqT = np.ascontiguousarray(xb[half * SQ:(half + 1) * SQ].T.astype(BF))  # [C, SQ]
        xk = np.zeros((L, C), dtype=np.float32)
        xk[:cnts[b]] = xb[keeps[b]]
        xkT = np.ascontiguousarray(xk.T.astype(BF))  # [C, L]
        mb1d = np.full(L, NEG, dtype=np.float32)
        mb1d[:cnts[b]] = 0.0
        mb = mb1d.reshape(KC, 128).T                 # [128, KC]
        mb2_1d = np.full(L, MB2_MASKED, dtype=np.float32)
        mb2_1d[:cnts[b]] = B_EXP
        mb2 = mb2_1d.reshape(KC, 128).T              # [128, KC]
        mbb = np.ascontiguousarray(np.concatenate([mb, mb2], axis=1))  # [128, 2KC]
        in_maps.append({
            "xqT": xqT, "xkT": xkT, "wqT": wqT, "wkT": wkT, "wvT": wvT,
            "w1T": w1T, "w2TA": w2TA, "mbb": mbb, "ident": ident,
            "bqk1": bqk1, "bvrow": bvrow,
        })
    return L, in_maps, use_bv, use_b


def kernel(x, mask, Wq, bq, Wk, bk, Wv, bv, W1, b1, W2, b2):
    L, in_maps, use_bv, use_b = _prep_inputs(x, mask, Wq, bq, Wk, bk, Wv, bv, W1, b1, W2, b2)
    key = (L, use_bv, use_b)
    if key not in _cache:
        _cache[key] = _build(L, use_bv, use_b)
    nc = _cache[key]
    res = None
    last_exc = None
    for attempt in range(4):
        try:
            res = run_bass_kernel_spmd(nc, in_maps, core_ids=list(range(NCORES)),
                                       trace=False)
            break
        except Exception as e:  # transient device errors on first exec of a NEFF
            last_exc = e
            import time as _time
            import jax as _jax
            _time.sleep(2.0)
            try:
                _jax.clear_caches()
            except Exception:
                pass
    if res is None:
        raise last_exc
    out = np.empty((B, S, C), dtype=np.float32)
    for core in range(NCORES):
        b, half = core // 2, core % 2
        out[b, half * SQ:(half + 1) * SQ] = res.results[core]["out"]
    if np.any(b2 != 0):
        out += np.asarray(b2, dtype=np.float32)[None, None, :]
    # stash for test harness reuse (timing reruns)
    kernel.last = {"nc": nc, "in_maps": in_maps, "L": L}
    return out


# revision 10
# speedup vs baseline: 1.5157x; 1.1629x over previous
"""Trainium2 Bass kernel for nn_Block_softmoe (dense transformer block, B=4 S=2048 C=256 H=8).

Strategy (v3)
-------------
Sharding: 8 cores = (batch b, query-half). Each core computes the full block for
1024 query rows of one batch. K/V are computed per-core over that batch's keys
(2x redundant K/V projection; tiny at dim 256). No collectives.

Mask compaction: the key mask (Bernoulli 0/1) is applied on the host by
gathering only the kept key rows (~1024 of 2048), so L ~= 1024, KC = L/128.

Cost-model facts this kernel is built around:
  - matmul cost = out-free-size x cycles_per_row (bf16/f32r>=256: 1.0); the
    stationary (lhsT) load is free -> stream the SMALL operand.
  - only ACT has exp; DVE fakes it with the Schraudolph bit trick
    (int16(a*y+b) bitcast to bf16; int16 saturation gives -0.0 for masked
    keys), so the 64 exp tiles are SPLIT across ACT and DVE (Bresenham
    interleave, ACT-heavy since DVE carries more copy work).
  - every HWDGE dma_start serializes ~625ns on the single HWDGE queue ->
    consolidate input DMAs and push non-critical loads + half the output
    stores through the Pool engine's SWDGE path (Pool is otherwise idle;
    GPSIMD cannot touch PSUM so it only gets SBUF/DRAM work).

Dataflow per core (SQ=1024 queries, L keys):
  QT = WqT.T @ xqT   [256, SQ] f32r (feature-major)   KT likewise [256, L]
  Vone[kc] [128, 8*33] bf16: per head h cols h*33..h*33+31 = V feats, +1 ones
  scores (h,kc): psum[128, SQ] = KT_h[kc].T @ QT_h   (PE, streams queries)
  P[h,kc] = exp(scale*scores + maskbias) -> bf16     (ACT exp | DVE bit trick)
  attnV (qc: 128-query chunk): po[128, 264] += P[h,kc][:,qc].T @ Vone[kc][h]
    -> ONE 33-wide stream gives attn.V AND the softmax denominator (PE).
    NOTE start=True pending-zeroes the whole PSUM bank, so only the first
    write into the po bank sets it.
  normalize: rec = 1/po[:,:,32] (DVE), xout[q,c] = po * rec (DVE, bcast AP)
  transpose xout -> feature-major xoutT via PE identity-transpose
  MLP in 256-col quarter strips (overlaps the attnV tail):
  h1T = gelu(W1T.T @ xoutT + b1) (ACT), final = [h1T;xoutT].T @ [W2T;I]
  (residual fused via identity block), out copies ACT -> DMA (SP/Pool split).

attnV for the first two query chunks is interleaved into the scores phase
(PSUM banks: 4 scores double-buffer + 2 po + 2 proj/mlp = 8).

Self-contained: hardcodes all shapes; compiled NEFF cached per L.
"""

import os
import sys

for _p in ("/opt/trn_rl_repo", "/root/.axon_site/_ro/trn_rl_repo"):
    if os.path.isdir(_p) and _p not in sys.path:
        sys.path.append(_p)

import ml_dtypes
import numpy as np

import concourse.bacc as bacc
import concourse.tile as tile
from concourse import mybir
from concourse.bass_utils import run_bass_kernel_spmd

B, S, C, H, HD = 4, 2048, 256, 8, 32
NCORES = 8
SQ = 1024                      # query rows per core
NQC = SQ // 128                # query chunks for attnV
SCALE = float(HD) ** -0.5
F32 = mybir.dt.float32
F32R = mybir.dt.float32r
BF16 = mybir.dt.bfloat16
I16 = mybir.dt.int16
AF = mybir.ActivationFunctionType
ALU = mybir.AluOpType
NEG = -1e30

# Schraudolph exp in bf16 bits: int16(A*y + B) viewed as bf16 ~= exp(y).
A_EXP = 2.0 ** 7 / np.log(2.0)
B_EXP = 127.0 * 2.0 ** 7 - 4.7
MB2_MASKED = -1e6              # saturates the int16 -> -32768 -> bf16 -0.0

N_DVE_EXP = 29                 # of the 64 exp units, how many go to DVE

_cache: dict = {}


def _build(L: int, use_bv: bool, use_b: bool = True):
    """Build the single-core program (SPMD across 8 cores)."""
    KC = L // 128
    nc = bacc.Bacc("TRN2", target_bir_lowering=False, debug=False, num_devices=NCORES)

    # ---- I/O ----
    d_xqT = nc.dram_tensor("xqT", [C, SQ], BF16, kind="ExternalInput")
    d_xkT = nc.dram_tensor("xkT", [C, L], BF16, kind="ExternalInput")
    d_wqT = nc.dram_tensor("wqT", [C, C], BF16, kind="ExternalInput")
    d_wkT = nc.dram_tensor("wkT", [C, C], BF16, kind="ExternalInput")
    d_wvT = nc.dram_tensor("wvT", [C, C], BF16, kind="ExternalInput")
    d_w1T = nc.dram_tensor("w1T", [C, C], F32R, kind="ExternalInput")
    d_w2TA = nc.dram_tensor("w2TA", [2 * C, C], F32R, kind="ExternalInput")
    d_mbb = nc.dram_tensor("mbb", [128, 2 * KC], F32, kind="ExternalInput")  # mb|mb2
    d_ident = nc.dram_tensor("ident", [128, 128], F32R, kind="ExternalInput")
    d_bqk1 = nc.dram_tensor("bqk1", [128, 6], F32, kind="ExternalInput")  # bq|bk|b1
    d_bvrow = nc.dram_tensor("bvrow", [1, C], F32R, kind="ExternalInput")
    d_out = nc.dram_tensor("out", [SQ, C], F32, kind="ExternalOutput")

    # DVE/ACT exp assignment: Bresenham spread of N_DVE_EXP Ds over 8*KC units
    nu = 8 * KC
    nd = min(N_DVE_EXP, nu)
    dve_units = {u for u in range(nu)
                 if (u * nd) // nu != ((u + 1) * nd) // nu}

    with tile.TileContext(nc) as tc:
        with tc.tile_pool(name="persist", bufs=1) as pp, \
             tc.tile_pool(name="pt", bufs=1) as ptp, \
             tc.tile_pool(name="work", bufs=3) as wp, \
             tc.tile_pool(name="ps_r", bufs=3, space="PSUM") as ps_r, \
             tc.tile_pool(name="ps_po", bufs=2, space="PSUM") as ps_po:

            # ---- consolidated tiles (chunk-major columns) ----
            xqT = pp.tile([128, 2 * SQ], BF16, tag="xqT", name="xqT")    # cols kk*SQ+q
            xkT = pp.tile([128, 2 * L], BF16, tag="xkT", name="xkT")     # cols kk*L+t
            wqT = pp.tile([128, 2 * C], BF16, tag="wqT", name="wqT")     # cols kk*C+f
            wkT = pp.tile([128, 2 * C], BF16, tag="wkT", name="wkT")
            wvT = pp.tile([128, 2 * C], BF16, tag="wvT", name="wvT")
            w1T = pp.tile([128, 2 * C], F32R, tag="w1T", name="w1T")
            w2TA = pp.tile([128, 4 * C], F32R, tag="w2TA", name="w2TA")  # cols cc*C+f
            mbb = pp.tile([128, 2 * KC], F32, tag="mbb", name="mbb")
            ident = pp.tile([128, 128], F32R, tag="ident", name="ident")

            def _chunks(dram, tile_t, nch, w, dt_):
                # one DMA: DRAM [nch*128, w] -> SBUF [128, nch*w] chunk-major
                nc_ = dram[:, :].rearrange("(c p) w -> p c w", c=nch)
                nc.sync.dma_start(out=tile_t[:, :].rearrange("p (c w) -> p c w", c=nch),
                                  in_=nc_)

            # critical path on HWDGE: weights first (tiny), then x
            _chunks(d_wqT, wqT, 2, C, BF16)
            _chunks(d_wkT, wkT, 2, C, BF16)
            _chunks(d_xqT, xqT, 2, SQ, BF16)
            _chunks(d_xkT, xkT, 2, L, BF16)
            # bulk loads via Pool SWDGE (off the HWDGE queue)
            nc.gpsimd.dma_start(out=mbb, in_=d_mbb[:, :])
            nc.gpsimd.dma_start(out=wvT[:, :].rearrange("p (c w) -> p c w", c=2),
                                in_=d_wvT[:, :].rearrange("(c p) w -> p c w", c=2))
            if use_b:
                bqk1 = pp.tile([128, 6], F32, tag="bqk1", name="bqk1")
                nc.gpsimd.dma_start(out=bqk1, in_=d_bqk1[:, :])
            nc.gpsimd.dma_start(out=ident, in_=d_ident[:, :])
            nc.gpsimd.dma_start(out=w1T[:, :].rearrange("p (c w) -> p c w", c=2),
                                in_=d_w1T[:, :].rearrange("(c p) w -> p c w", c=2))
            nc.gpsimd.dma_start(out=w2TA[:, :].rearrange("p (c w) -> p c w", c=4),
                                in_=d_w2TA[:, :].rearrange("(c p) w -> p c w", c=4))
            if use_bv:
                bvrow = pp.tile([1, C], F32R, tag="bvrow", name="bvrow")
                onesr = pp.tile([1, 128], F32R, tag="onesr", name="onesr")
                nc.gpsimd.dma_start(out=bvrow, in_=d_bvrow[:, :])
                nc.vector.memset(onesr, 1.0)

            mb = mbb[:, 0:KC]
            mb2 = mbb[:, KC:2 * KC]

            # ---- persistent intermediates ----
            QT = [pp.tile([128, SQ], F32R, tag=f"QT{m}", name=f"QT{m}") for m in range(2)]
            KT = [pp.tile([128, L], F32R, tag=f"KT{m}", name=f"KT{m}") for m in range(2)]
            Vone = [pp.tile([128, H * 33], BF16, tag=f"Vone{sc}", name=f"Vone{sc}")
                    for sc in range(KC)]
            # feature-major attn output: cols = cc*SQ + q
            xoutT = pp.tile([128, 2 * SQ], F32R, tag="xoutT", name="xoutT")
            h1T = [pp.tile([128, SQ], F32R, tag=f"h1T{j}", name=f"h1T{j}") for j in range(2)]
            PT = {}

            def _ps():
                # one unified PSUM ring (3 x [128,1024] = 6 banks) shared by
                # proj / scores / transpose / MLP; users slice what they need
                return ps_r.tile([128, SQ], F32, tag="ring", name="ring")

            kchunks = [(o, min(512, L - o)) for o in range(0, L, 512)]

            def emit_qk_proj(m):
                for n in range(2):  # Q: SQ/512
                    pq = _ps()
                    for kk in range(2):
                        nc.tensor.matmul(out=pq[:, 0:512], lhsT=wqT[:, kk * C + m * 128:kk * C + (m + 1) * 128],
                                         rhs=xqT[:, kk * SQ + n * 512:kk * SQ + (n + 1) * 512],
                                         start=(kk == 0), stop=(kk == 1))
                    if use_b:
                        nc.vector.tensor_scalar_add(out=QT[m][:, n * 512:(n + 1) * 512],
                                                    in0=pq[:, 0:512], scalar1=bqk1[:, m:m + 1])
                    else:
                        nc.scalar.copy(out=QT[m][:, n * 512:(n + 1) * 512], in_=pq[:, 0:512])
                for o, w in kchunks:
                    pk = _ps()
                    for kk in range(2):
                        nc.tensor.matmul(out=pk[:, 0:w], lhsT=wkT[:, kk * C + m * 128:kk * C + (m + 1) * 128],
                                         rhs=xkT[:, kk * L + o:kk * L + o + w],
                                         start=(kk == 0), stop=(kk == 1))
                    if use_b:
                        nc.vector.tensor_scalar_add(out=KT[m][:, o:o + w], in0=pk[:, 0:w],
                                                    scalar1=bqk1[:, 2 + m:3 + m])
                    else:
                        nc.vector.tensor_copy(out=KT[m][:, o:o + w], in_=pk[:, 0:w])

            def emit_v_proj(sc):
                pv = _ps()
                for kk in range(2):
                    nc.tensor.matmul(out=pv[:, :C],
                                     lhsT=xkT[:, kk * L + sc * 128:kk * L + (sc + 1) * 128],
                                     rhs=wvT[:, kk * C:(kk + 1) * C], start=(kk == 0),
                                     stop=(kk == 1) and not use_bv)
                if use_bv:
                    nc.tensor.matmul(out=pv[:, :C], lhsT=onesr[0:1, :],
                                     rhs=bvrow[0:1, :], start=False, stop=True)
                vr = Vone[sc][:, :].rearrange("p (h w) -> p h w", h=H)
                nc.vector.tensor_copy(out=vr[:, :, 0:32],
                                      in_=pv[:, :C].rearrange("p (h w) -> p h w", h=H))
                nc.gpsimd.memset(vr[:, :, 32:33], 1.0)

            uidx = [0]

            def emit_score_exp(h, kc):
                g, j = h // 4, h % 4
                pss = _ps()
                for qn in range(2):
                    nc.tensor.matmul(
                        out=pss[:, qn * 512:(qn + 1) * 512],
                        lhsT=KT[g][32 * j:32 * j + 32, kc * 128:(kc + 1) * 128],
                        rhs=QT[g][32 * j:32 * j + 32, qn * 512:(qn + 1) * 512],
                        start=True, stop=True,
                        tile_position=(32 * j, 0))
                pt_t = ptp.tile([128, SQ], BF16, tag="pt", bufs=8 * KC,
                                name=f"pt{h}_{kc}")
                if uidx[0] in dve_units:
                    nc.vector.tensor_scalar(out=pt_t.bitcast(I16), in0=pss,
                                            scalar1=float(SCALE * A_EXP),
                                            scalar2=mb2[:, kc:kc + 1],
                                            op0=ALU.mult, op1=ALU.add)
                else:
                    nc.scalar.activation(out=pt_t, in_=pss, func=AF.Exp,
                                         bias=mb[:, kc:kc + 1], scale=SCALE)
                uidx[0] += 1
                PT[h, kc] = pt_t

            po_of = {}

            def emit_attn(qc, kc):
                if kc == 0:
                    po_of[qc] = ps_po.tile([128, H * 33], F32, tag="po", name=f"po{qc}")
                po = po_of[qc]
                for h in range(H):
                    # start=True pending-zeroes the WHOLE psum bank, so only
                    # the very first write into the bank may set it.
                    nc.tensor.matmul(
                        out=po[:, h * 33:(h + 1) * 33],
                        lhsT=PT[h, kc][:, qc * 128:(qc + 1) * 128],
                        rhs=Vone[kc][:, h * 33:(h + 1) * 33],
                        start=(kc == 0 and h == 0), stop=(kc == KC - 1),
                        skip_group_check=(h > 0))

            def emit_norm_transpose(qc):
                po = po_of[qc][:, :].rearrange("p (h w) -> p h w", h=H)
                rec = wp.tile([128, H, 1], F32, tag="rec", name="rec")
                nc.vector.reciprocal(out=rec, in_=po[:, :, 32:33])
                xo = wp.tile([128, C], F32R, tag="xo", name="xo")
                nc.vector.tensor_mul(out=xo[:, :].rearrange("p (h w) -> p h w", h=H),
                                     in0=po[:, :, 0:32],
                                     in1=rec[:, :, :].broadcast_to((128, H, 32)))
                ptr = _ps()
                for cc in range(2):
                    nc.tensor.transpose(out=ptr[:, cc * 128:(cc + 1) * 128].bitcast(F32R),
                                        in_=xo[:, cc * 128:(cc + 1) * 128],
                                        identity=ident)
                xoT = xoutT[:, :].rearrange("p (c q) -> p c q", c=2)
                nc.vector.tensor_copy(
                    out=xoT[:, :, qc * 128:(qc + 1) * 128],
                    in_=ptr[:, :C].rearrange("p (c q) -> p c q", c=2).bitcast(F32R))

            def emit_mlp_quarter(n):
                # 256-col strip: queries n*256..(n+1)*256 (query chunks 2n, 2n+1)
                for j in range(2):
                    ph = _ps()
                    for cc in range(2):
                        nc.tensor.matmul(out=ph[:, 0:256], lhsT=w1T[:, cc * C + j * 128:cc * C + (j + 1) * 128],
                                         rhs=xoutT[:, cc * SQ + n * 256:cc * SQ + (n + 1) * 256],
                                         start=(cc == 0), stop=(cc == 1))
                    nc.scalar.activation(out=h1T[j][:, n * 256:(n + 1) * 256], in_=ph[:, 0:256],
                                         func=AF.Gelu,
                                         bias=(bqk1[:, 4 + j:5 + j] if use_b else 0.0))
                for sc in range(2 * n, 2 * n + 2):
                    pf = _ps()
                    for cc in range(4):
                        lh = (h1T[cc][:, sc * 128:(sc + 1) * 128] if cc < 2 else
                              xoutT[:, (cc - 2) * SQ + sc * 128:(cc - 2) * SQ + (sc + 1) * 128])
                        nc.tensor.matmul(out=pf[:, :C], lhsT=lh, rhs=w2TA[:, cc * C:(cc + 1) * C],
                                         start=(cc == 0), stop=(cc == 3))
                    ot = wp.tile([128, C], F32, tag="ot", name="ot")
                    nc.scalar.copy(out=ot, in_=pf[:, :C])
                    if sc % 2 == 0:
                        nc.sync.dma_start(out=d_out[sc * 128:(sc + 1) * 128, :], in_=ot)
                    else:
                        nc.gpsimd.dma_start(out=d_out[sc * 128:(sc + 1) * 128, :], in_=ot)

            # ---- emission schedule ----
            emit_qk_proj(0)
            for h in range(4):
                emit_score_exp(h, 0)
            emit_qk_proj(1)
            for h in range(4, 8):
                emit_score_exp(h, 0)
            for sc in range(min(4, KC)):
                emit_v_proj(sc)
            if KC > 1:
                for h in range(H):
                    emit_score_exp(h, 1)
            for sc in range(4, KC):
                emit_v_proj(sc)
            # scores kc>=2, with attnV for qc 0/1 interleaved 3 batches behind
            for kc in range(2, KC):
                for h in range(H):
                    emit_score_exp(h, kc)
                akc = kc - 3
                if 0 <= akc <= KC - 4:
                    emit_attn(0, akc)
                    emit_attn(1, akc)
            # tail: finish qc0/qc1, then remaining chunks; MLP quarter n
            # follows query-chunk 2n+1
            for akc in range(max(0, KC - 3), KC):
                emit_attn(0, akc)
                emit_attn(1, akc)
            emit_norm_transpose(0)
            emit_norm_transpose(1)
            emit_mlp_quarter(0)
            for qc in range(2, NQC):
                for kc in range(KC):
                    emit_attn(qc, kc)
                emit_norm_transpose(qc)
                if qc % 2 == 1:
                    emit_mlp_quarter(qc // 2)

    nc.compile()
    return nc


def _prep_inputs(x, mask, Wq, bq, Wk, bk, Wv, bv, W1, b1, W2, b2):
    """Host-side sharding + layout prep. Returns (L, in_maps, use_bv, use_b)."""
    x = np.ascontiguousarray(x, dtype=np.float32)
    keeps = [np.flatnonzero(mask[b, :S] != 0) for b in range(B)]
    cnts = [len(k) for k in keeps]
    L = max(128, -(-max(cnts) // 128) * 128)
    KC = L // 128

    BF = ml_dtypes.bfloat16
    wqT = np.ascontiguousarray(np.asarray(Wq, np.float32).T.astype(BF))
    wkT = np.ascontiguousarray(np.asarray(Wk, np.float32).T.astype(BF))
    wvT = np.ascontiguousarray(np.asarray(Wv, np.float32).T.astype(BF))
    w1T = np.ascontiguousarray(W1.T, dtype=np.float32)
    w2TA = np.ascontiguousarray(
        np.vstack([W2.T.astype(np.float32), np.eye(C, dtype=np.float32)]))
    ident = np.eye(128, dtype=np.float32)
    bqk1 = np.stack([
        bq[0:128], bq[128:256], bk[0:128], bk[128:256], b1[0:128], b1[128:256],
    ], axis=1).astype(np.float32)
    bvrow = np.asarray(bv, np.float32).reshape(1, C)
    use_bv = bool(np.any(bv != 0))
    use_b = bool(np.any(bq != 0) or np.any(bk != 0) or np.any(b1 != 0))

    in_maps = []
    for core in range(NCORES):
        b, half = core // 2, core % 2
        xb = x[b]                                   # [S, C]
        xqT = np.ascontiguousarray(xb[half * SQ:(half + 1) * SQ].T.astype(BF))  # [C, SQ]
        xk = np.zeros((L, C), dtype=np.float32)
        xk[:cnts[b]] = xb[keeps[b]]
        xkT = np.ascontiguousarray(xk.T.astype(BF))  # [C, L]
        mb1d = np.full(L, NEG, dtype=np.float32)
        mb1d[:cnts[b]] = 0.0
        mb = mb1d.reshape(KC, 128).T                 # [128, KC]
        mb2_1d = np.full(L, MB2_MASKED, dtype=np.float32)
        mb2_1d[:cnts[b]] = B_EXP
        mb2 = mb2_1d.reshape(KC, 128).T              # [128, KC]
        mbb = np.ascontiguousarray(np.concatenate([mb, mb2], axis=1))  # [128, 2KC]
        in_maps.append({
            "xqT": xqT, "xkT": xkT, "wqT": wqT, "wkT": wkT, "wvT": wvT,
            "w1T": w1T, "w2TA": w2TA, "mbb": mbb, "ident": ident,
            "bqk1": bqk1, "bvrow": bvrow,
        })
    return L, in_maps, use_bv, use_b


def kernel(x, mask, Wq, bq, Wk, bk, Wv, bv, W1, b1, W2, b2):
    L, in_maps, use_bv, use_b = _prep_inputs(x, mask, Wq, bq, Wk, bk, Wv, bv, W1, b1, W2, b2)
    key = (L, use_bv, use_b)
    if key not in _cache:
        _cache[key] = _build(L, use_bv, use_b)
    nc = _cache[key]
    res = None
    last_exc = None
    for attempt in range(4):
        try:
            res = run_bass_kernel_spmd(nc, in_maps, core_ids=list(range(NCORES)),
                                       trace=False)
            break
        except Exception as e:  # transient device errors on first exec of a NEFF
            last_exc = e
            import time as _time
            import jax as _jax
            _time.sleep(2.0)
            try:
                _jax.clear_caches()
            except Exception:
                pass
    if res is None:
        raise last_exc
    out = np.empty((B, S, C), dtype=np.float32)
    for core in range(NCORES):
        b, half = core // 2, core % 2
        out[b, half * SQ:(half + 1) * SQ] = res.results[core]["out"]
    if np.any(b2 != 0):
        out += np.asarray(b2, dtype=np.float32)[None, None, :]
    # stash for test harness reuse (timing reruns)
    kernel.last = {"nc": nc, "in_maps": in_maps, "L": L}
    return out


# revision 12
# speedup vs baseline: 1.5868x; 1.0470x over previous
"""Trainium2 Bass kernel for nn_Block_softmoe (dense transformer block, B=4 S=2048 C=256 H=8).

Strategy (v3)
-------------
Sharding: 8 cores = (batch b, query-half). Each core computes the full block for
1024 query rows of one batch. K/V are computed per-core over that batch's keys
(2x redundant K/V projection; tiny at dim 256). No collectives.

Mask compaction: the key mask (Bernoulli 0/1) is applied on the host by
gathering only the kept key rows (~1024 of 2048), so L ~= 1024, KC = L/128.

Cost-model facts this kernel is built around:
  - matmul cost = out-free-size x cycles_per_row (bf16/f32r>=256: 1.0); the
    stationary (lhsT) load is free -> stream the SMALL operand.
  - only ACT has exp; DVE fakes it with the Schraudolph bit trick
    (int16(a*y+b) bitcast to bf16; int16 saturation gives -0.0 for masked
    keys), so the 64 exp tiles are SPLIT across ACT and DVE (Bresenham
    interleave, ACT-heavy since DVE carries more copy work).
  - every HWDGE dma_start serializes ~625ns on the single HWDGE queue ->
    consolidate input DMAs and push non-critical loads + half the output
    stores through the Pool engine's SWDGE path (Pool is otherwise idle;
    GPSIMD cannot touch PSUM so it only gets SBUF/DRAM work).

Dataflow per core (SQ=1024 queries, L keys):
  QT = WqT.T @ xqT   [256, SQ] f32r (feature-major)   KT likewise [256, L]
  Vone[kc] [128, 8*33] bf16: per head h cols h*33..h*33+31 = V feats, +1 ones
  scores (h,kc): psum[128, SQ] = KT_h[kc].T @ QT_h   (PE, streams queries)
  P[h,kc] = exp(scale*scores + maskbias) -> bf16     (ACT exp | DVE bit trick)
  attnV (qc: 128-query chunk): po[128, 264] += P[h,kc][:,qc].T @ Vone[kc][h]
    -> ONE 33-wide stream gives attn.V AND the softmax denominator (PE).
    NOTE start=True pending-zeroes the whole PSUM bank, so only the first
    write into the po bank sets it.
  normalize: rec = 1/po[:,:,32] (DVE), xout[q,c] = po * rec (DVE, bcast AP)
  transpose xout -> feature-major xoutT via PE identity-transpose
  MLP in 256-col quarter strips (overlaps the attnV tail):
  h1T = gelu(W1T.T @ xoutT + b1) (ACT), final = [h1T;xoutT].T @ [W2T;I]
  (residual fused via identity block), out copies ACT -> DMA (SP/Pool split).

attnV for the first two query chunks is interleaved into the scores phase
(PSUM banks: 4 scores double-buffer + 2 po + 2 proj/mlp = 8).

Self-contained: hardcodes all shapes; compiled NEFF cached per L.
"""

import os
import sys

for _p in ("/opt/trn_rl_repo", "/root/.axon_site/_ro/trn_rl_repo"):
    if os.path.isdir(_p) and _p not in sys.path:
        sys.path.append(_p)

import ml_dtypes
import numpy as np

import concourse.bacc as bacc
import concourse.tile as tile
from concourse import mybir
from concourse.bass_utils import run_bass_kernel_spmd

B, S, C, H, HD = 4, 2048, 256, 8, 32
NCORES = 8
SQ = 1024                      # query rows per core
NQC = SQ // 128                # query chunks for attnV
SCALE = float(HD) ** -0.5
F32 = mybir.dt.float32
F32R = mybir.dt.float32r
BF16 = mybir.dt.bfloat16
I16 = mybir.dt.int16
AF = mybir.ActivationFunctionType
ALU = mybir.AluOpType
NEG = -1e30

# Schraudolph exp in bf16 bits: int16(A*y + B) viewed as bf16 ~= exp(y).
A_EXP = 2.0 ** 7 / np.log(2.0)
B_EXP = 127.0 * 2.0 ** 7 - 4.7
MB2_MASKED = -1e6              # saturates the int16 -> -32768 -> bf16 -0.0

N_DVE_EXP = 29                 # of the 64 exp units, how many go to DVE

_cache: dict = {}


def _build(L: int, use_bv: bool, use_b: bool = True):
    """Build the single-core program (SPMD across 8 cores)."""
    KC = L // 128
    nc = bacc.Bacc("TRN2", target_bir_lowering=False, debug=False, num_devices=NCORES)

    # ---- I/O ----
    d_xqT = nc.dram_tensor("xqT", [C, SQ], BF16, kind="ExternalInput")
    d_xkT = nc.dram_tensor("xkT", [C, L], BF16, kind="ExternalInput")
    d_wqT = nc.dram_tensor("wqT", [C, C], BF16, kind="ExternalInput")
    d_wkT = nc.dram_tensor("wkT", [C, C], BF16, kind="ExternalInput")
    d_wvT = nc.dram_tensor("wvT", [C, C], BF16, kind="ExternalInput")
    d_w1T = nc.dram_tensor("w1T", [C, C], F32R, kind="ExternalInput")
    d_w2TA = nc.dram_tensor("w2TA", [2 * C, C], F32R, kind="ExternalInput")
    d_mbb = nc.dram_tensor("mbb", [128, 2 * KC], F32, kind="ExternalInput")  # mb|mb2
    d_ident = nc.dram_tensor("ident", [128, 128], F32R, kind="ExternalInput")
    d_bqk1 = nc.dram_tensor("bqk1", [128, 6], F32, kind="ExternalInput")  # bq|bk|b1
    d_bvrow = nc.dram_tensor("bvrow", [1, C], F32R, kind="ExternalInput")
    d_out = nc.dram_tensor("out", [SQ, C], F32, kind="ExternalOutput")

    # DVE/ACT exp assignment: Bresenham spread of N_DVE_EXP Ds over 8*KC units
    nu = 8 * KC
    nd = min(N_DVE_EXP, nu)
    dve_units = {u for u in range(nu)
                 if (u * nd) // nu != ((u + 1) * nd) // nu}

    with tile.TileContext(nc) as tc:
        with tc.tile_pool(name="persist", bufs=1) as pp, \
             tc.tile_pool(name="pt", bufs=1) as ptp, \
             tc.tile_pool(name="work", bufs=3) as wp, \
             tc.tile_pool(name="ps_r", bufs=4, space="PSUM") as ps_r:

            # ---- consolidated tiles (chunk-major columns) ----
            xqT = pp.tile([128, 2 * SQ], BF16, tag="xqT", name="xqT")    # cols kk*SQ+q
            xkT = pp.tile([128, 2 * L], BF16, tag="xkT", name="xkT")     # cols kk*L+t
            wqT = pp.tile([128, 2 * C], BF16, tag="wqT", name="wqT")     # cols kk*C+f
            wkT = pp.tile([128, 2 * C], BF16, tag="wkT", name="wkT")
            wvT = pp.tile([128, 2 * C], BF16, tag="wvT", name="wvT")
            w1T = pp.tile([128, 2 * C], F32R, tag="w1T", name="w1T")
            w2TA = pp.tile([128, 4 * C], F32R, tag="w2TA", name="w2TA")  # cols cc*C+f
            mbb = pp.tile([128, 2 * KC], F32, tag="mbb", name="mbb")
            ident = pp.tile([128, 128], F32R, tag="ident", name="ident")

            def _chunks(dram, tile_t, nch, w, dt_):
                # one DMA: DRAM [nch*128, w] -> SBUF [128, nch*w] chunk-major
                nc_ = dram[:, :].rearrange("(c p) w -> p c w", c=nch)
                nc.sync.dma_start(out=tile_t[:, :].rearrange("p (c w) -> p c w", c=nch),
                                  in_=nc_)

            # critical path on HWDGE: weights first (tiny), then x
            _chunks(d_wqT, wqT, 2, C, BF16)
            _chunks(d_wkT, wkT, 2, C, BF16)
            _chunks(d_xqT, xqT, 2, SQ, BF16)
            _chunks(d_xkT, xkT, 2, L, BF16)
            # bulk loads via Pool SWDGE (off the HWDGE queue)
            nc.gpsimd.dma_start(out=mbb, in_=d_mbb[:, :])
            nc.gpsimd.dma_start(out=wvT[:, :].rearrange("p (c w) -> p c w", c=2),
                                in_=d_wvT[:, :].rearrange("(c p) w -> p c w", c=2))
            if use_b:
                bqk1 = pp.tile([128, 6], F32, tag="bqk1", name="bqk1")
                nc.gpsimd.dma_start(out=bqk1, in_=d_bqk1[:, :])
            nc.gpsimd.dma_start(out=ident, in_=d_ident[:, :])
            nc.gpsimd.dma_start(out=w1T[:, :].rearrange("p (c w) -> p c w", c=2),
                                in_=d_w1T[:, :].rearrange("(c p) w -> p c w", c=2))
            nc.gpsimd.dma_start(out=w2TA[:, :].rearrange("p (c w) -> p c w", c=4),
                                in_=d_w2TA[:, :].rearrange("(c p) w -> p c w", c=4))
            if use_bv:
                bvrow = pp.tile([1, C], F32R, tag="bvrow", name="bvrow")
                onesr = pp.tile([1, 128], F32R, tag="onesr", name="onesr")
                nc.gpsimd.dma_start(out=bvrow, in_=d_bvrow[:, :])
                nc.vector.memset(onesr, 1.0)

            mb = mbb[:, 0:KC]
            mb2 = mbb[:, KC:2 * KC]

            # ---- persistent intermediates ----
            QT = [pp.tile([128, SQ], F32R, tag=f"QT{m}", name=f"QT{m}") for m in range(2)]
            KT = [pp.tile([128, L], F32R, tag=f"KT{m}", name=f"KT{m}") for m in range(2)]
            Vone = [pp.tile([128, H * 33], BF16, tag=f"Vone{sc}", name=f"Vone{sc}")
                    for sc in range(KC)]
            # feature-major attn output: cols = cc*SQ + q
            xoutT = pp.tile([128, 2 * SQ], F32R, tag="xoutT", name="xoutT")
            h1T = [pp.tile([128, SQ], F32R, tag=f"h1T{j}", name=f"h1T{j}") for j in range(2)]
            PT = {}

            def _ps():
                # one unified PSUM ring (3 x [128,1024] = 6 banks) shared by
                # proj / scores / transpose / MLP; users slice what they need
                return ps_r.tile([128, SQ], F32, tag="ring", name="ring")

            kchunks = [(o, min(512, L - o)) for o in range(0, L, 512)]

            def emit_qk_proj(m):
                for n in range(2):  # Q: SQ/512
                    pq = _ps()
                    for kk in range(2):
                        nc.tensor.matmul(out=pq[:, 0:512], lhsT=wqT[:, kk * C + m * 128:kk * C + (m + 1) * 128],
                                         rhs=xqT[:, kk * SQ + n * 512:kk * SQ + (n + 1) * 512],
                                         start=(kk == 0), stop=(kk == 1))
                    if use_b:
                        nc.vector.tensor_scalar_add(out=QT[m][:, n * 512:(n + 1) * 512],
                                                    in0=pq[:, 0:512], scalar1=bqk1[:, m:m + 1])
                    else:
                        nc.scalar.copy(out=QT[m][:, n * 512:(n + 1) * 512], in_=pq[:, 0:512])
                for o, w in kchunks:
                    pk = _ps()
                    for kk in range(2):
                        nc.tensor.matmul(out=pk[:, 0:w], lhsT=wkT[:, kk * C + m * 128:kk * C + (m + 1) * 128],
                                         rhs=xkT[:, kk * L + o:kk * L + o + w],
                                         start=(kk == 0), stop=(kk == 1))
                    if use_b:
                        nc.vector.tensor_scalar_add(out=KT[m][:, o:o + w], in0=pk[:, 0:w],
                                                    scalar1=bqk1[:, 2 + m:3 + m])
                    else:
                        nc.vector.tensor_copy(out=KT[m][:, o:o + w], in_=pk[:, 0:w])

            def emit_v_proj(sc):
                pv = _ps()
                for kk in range(2):
                    nc.tensor.matmul(out=pv[:, :C],
                                     lhsT=xkT[:, kk * L + sc * 128:kk * L + (sc + 1) * 128],
                                     rhs=wvT[:, kk * C:(kk + 1) * C], start=(kk == 0),
                                     stop=(kk == 1) and not use_bv)
                if use_bv:
                    nc.tensor.matmul(out=pv[:, :C], lhsT=onesr[0:1, :],
                                     rhs=bvrow[0:1, :], start=False, stop=True)
                vr = Vone[sc][:, :].rearrange("p (h w) -> p h w", h=H)
                nc.vector.tensor_copy(out=vr[:, :, 0:32],
                                      in_=pv[:, :C].rearrange("p (h w) -> p h w", h=H))
                nc.gpsimd.memset(vr[:, :, 32:33], 1.0)

            uidx = [0]

            def emit_score_exp(h, kc):
                g, j = h // 4, h % 4
                pss = _ps()
                for qn in range(2):
                    nc.tensor.matmul(
                        out=pss[:, qn * 512:(qn + 1) * 512],
                        lhsT=KT[g][32 * j:32 * j + 32, kc * 128:(kc + 1) * 128],
                        rhs=QT[g][32 * j:32 * j + 32, qn * 512:(qn + 1) * 512],
                        start=True, stop=True,
                        tile_position=(32 * j, 0))
                pt_t = ptp.tile([128, SQ], BF16, tag="pt", bufs=8 * KC,
                                name=f"pt{h}_{kc}")
                if uidx[0] in dve_units:
                    nc.vector.tensor_scalar(out=pt_t.bitcast(I16), in0=pss,
                                            scalar1=float(SCALE * A_EXP),
                                            scalar2=mb2[:, kc:kc + 1],
                                            op0=ALU.mult, op1=ALU.add)
                else:
                    nc.scalar.activation(out=pt_t, in_=pss, func=AF.Exp,
                                         bias=mb[:, kc:kc + 1], scale=SCALE)
                uidx[0] += 1
                PT[h, kc] = pt_t

            po_of = {}

            def emit_attn(qc, kc):
                if kc == 0:
                    po_of[qc] = _ps()
                po = po_of[qc]
                for h in range(H):
                    # start=True pending-zeroes the WHOLE psum bank, so only
                    # the very first write into the bank may set it.
                    nc.tensor.matmul(
                        out=po[:, h * 33:(h + 1) * 33],
                        lhsT=PT[h, kc][:, qc * 128:(qc + 1) * 128],
                        rhs=Vone[kc][:, h * 33:(h + 1) * 33],
                        start=(kc == 0 and h == 0), stop=(kc == KC - 1),
                        skip_group_check=(h > 0))

            def emit_norm_transpose(qc):
                po = po_of[qc][:, 0:H * 33].rearrange("p (h w) -> p h w", h=H)
                rec = wp.tile([128, H, 1], F32, tag="rec", name="rec")
                nc.vector.reciprocal(out=rec, in_=po[:, :, 32:33])
                xo = wp.tile([128, C], F32R, tag="xo", name="xo")
                nc.vector.tensor_mul(out=xo[:, :].rearrange("p (h w) -> p h w", h=H),
                                     in0=po[:, :, 0:32],
                                     in1=rec[:, :, :].broadcast_to((128, H, 32)))
                ptr = _ps()
                for cc in range(2):
                    nc.tensor.transpose(out=ptr[:, cc * 128:(cc + 1) * 128].bitcast(F32R),
                                        in_=xo[:, cc * 128:(cc + 1) * 128],
                                        identity=ident)
                xoT = xoutT[:, :].rearrange("p (c q) -> p c q", c=2)
                nc.vector.tensor_copy(
                    out=xoT[:, :, qc * 128:(qc + 1) * 128],
                    in_=ptr[:, :C].rearrange("p (c q) -> p c q", c=2).bitcast(F32R))

            def emit_mlp_quarter(n):
                # 256-col strip: queries n*256..(n+1)*256 (query chunks 2n, 2n+1)
                for j in range(2):
                    ph = _ps()
                    for cc in range(2):
                        nc.tensor.matmul(out=ph[:, 0:256], lhsT=w1T[:, cc * C + j * 128:cc * C + (j + 1) * 128],
                                         rhs=xoutT[:, cc * SQ + n * 256:cc * SQ + (n + 1) * 256],
                                         start=(cc == 0), stop=(cc == 1))
                    nc.scalar.activation(out=h1T[j][:, n * 256:(n + 1) * 256], in_=ph[:, 0:256],
                                         func=AF.Gelu,
                                         bias=(bqk1[:, 4 + j:5 + j] if use_b else 0.0))
                for sc in range(2 * n, 2 * n + 2):
                    pf = _ps()
                    for cc in range(4):
                        lh = (h1T[cc][:, sc * 128:(sc + 1) * 128] if cc < 2 else
                              xoutT[:, (cc - 2) * SQ + sc * 128:(cc - 2) * SQ + (sc + 1) * 128])
                        nc.tensor.matmul(out=pf[:, :C], lhsT=lh, rhs=w2TA[:, cc * C:(cc + 1) * C],
                                         start=(cc == 0), stop=(cc == 3))
                    ot = wp.tile([128, C], F32, tag="ot", name="ot")
                    nc.scalar.copy(out=ot, in_=pf[:, :C])
                    if sc % 2 == 0:
                        nc.sync.dma_start(out=d_out[sc * 128:(sc + 1) * 128, :], in_=ot)
                    else:
                        nc.gpsimd.dma_start(out=d_out[sc * 128:(sc + 1) * 128, :], in_=ot)

            # ---- emission schedule ----
            emit_qk_proj(0)
            for h in range(4):
                emit_score_exp(h, 0)
            emit_qk_proj(1)
            for h in range(4, 8):
                emit_score_exp(h, 0)
            for sc in range(min(4, KC)):
                emit_v_proj(sc)
            if KC > 1:
                for h in range(H):
                    emit_score_exp(h, 1)
            for sc in range(4, KC):
                emit_v_proj(sc)
            for kc in range(2, KC):
                for h in range(H):
                    emit_score_exp(h, kc)
            # tail: attnV chains through the same psum ring; norm/transpose of
            # chain qc runs while chain qc+1 accumulates; MLP quarter n after
            # both its query chunks (2n, 2n+1) are normalized
            for qc in range(NQC):
                for kc in range(KC):
                    emit_attn(qc, kc)
                if qc >= 1:
                    emit_norm_transpose(qc - 1)
                if qc >= 3 and qc % 2 == 1:
                    emit_mlp_quarter((qc - 3) // 2)
            emit_norm_transpose(NQC - 1)
            emit_mlp_quarter(NQC // 2 - 1)

    nc.compile()
    return nc


def _prep_inputs(x, mask, Wq, bq, Wk, bk, Wv, bv, W1, b1, W2, b2):
    """Host-side sharding + layout prep. Returns (L, in_maps, use_bv, use_b)."""
    x = np.ascontiguousarray(x, dtype=np.float32)
    keeps = [np.flatnonzero(mask[b, :S] != 0) for b in range(B)]
    cnts = [len(k) for k in keeps]
    L = max(128, -(-max(cnts) // 128) * 128)
    KC = L // 128

    BF = ml_dtypes.bfloat16
    wqT = np.ascontiguousarray(np.asarray(Wq, np.float32).T.astype(BF))
    wkT = np.ascontiguousarray(np.asarray(Wk, np.float32).T.astype(BF))
    wvT = np.ascontiguousarray(np.asarray(Wv, np.float32).T.astype(BF))
    w1T = np.ascontiguousarray(W1.T, dtype=np.float32)
    w2TA = np.ascontiguousarray(
        np.vstack([W2.T.astype(np.float32), np.eye(C, dtype=np.float32)]))
    ident = np.eye(128, dtype=np.float32)
    bqk1 = np.stack([
        bq[0:128], bq[128:256], bk[0:128], bk[128:256], b1[0:128], b1[128:256],
    ], axis=1).astype(np.float32)
    bvrow = np.asarray(bv, np.float32).reshape(1, C)
    use_bv = bool(np.any(bv != 0))
    use_b = bool(np.any(bq != 0) or np.any(bk != 0) or np.any(b1 != 0))

    in_maps = []
    for core in range(NCORES):
        b, half = core // 2, core % 2
        xb = x[b]                                   # [S, C]
        xqT = np.ascontiguousarray(xb[half * SQ:(half + 1) * SQ].T.astype(BF))  # [C, SQ]
        xk = np.zeros((L, C), dtype=np.float32)
        xk[:cnts[b]] = xb[keeps[b]]
        xkT = np.ascontiguousarray(xk.T.astype(BF))  # [C, L]
        mb1d = np.full(L, NEG, dtype=np.float32)
        mb1d[:cnts[b]] = 0.0
        mb = mb1d.reshape(KC, 128).T                 # [128, KC]
        mb2_1d = np.full(L, MB2_MASKED, dtype=np.float32)
        mb2_1d[:cnts[b]] = B_EXP
        mb2 = mb2_1d.reshape(KC, 128).T              # [128, KC]
        mbb = np.ascontiguousarray(np.concatenate([mb, mb2], axis=1))  # [128, 2KC]
        in_maps.append({
            "xqT": xqT, "xkT": xkT, "wqT": wqT, "wkT": wkT, "wvT": wvT,
            "w1T": w1T, "w2TA": w2TA, "mbb": mbb, "ident": ident,
            "bqk1": bqk1, "bvrow": bvrow,
        })
    return L, in_maps, use_bv, use_b


def kernel(x, mask, Wq, bq, Wk, bk, Wv, bv, W1, b1, W2, b2):
    L, in_maps, use_bv, use_b = _prep_inputs(x, mask, Wq, bq, Wk, bk, Wv, bv, W1, b1, W2, b2)
    key = (L, use_bv, use_b)
    if key not in _cache:
        _cache[key] = _build(L, use_bv, use_b)
    nc = _cache[key]
    res = None
    last_exc = None
    for attempt in range(4):
        try:
            res = run_bass_kernel_spmd(nc, in_maps, core_ids=list(range(NCORES)),
                                       trace=False)
            break
        except Exception as e:  # transient device errors on first exec of a NEFF
            last_exc = e
            import time as _time
            import jax as _jax
            _time.sleep(2.0)
            try:
                _jax.clear_caches()
            except Exception:
                pass
    if res is None:
        raise last_exc
    out = np.empty((B, S, C), dtype=np.float32)
    for core in range(NCORES):
        b, half = core // 2, core % 2
        out[b, half * SQ:(half + 1) * SQ] = res.results[core]["out"]
    if np.any(b2 != 0):
        out += np.asarray(b2, dtype=np.float32)[None, None, :]
    # stash for test harness reuse (timing reruns)
    kernel.last = {"nc": nc, "in_maps": in_maps, "L": L}
    return out


# revision 14
# speedup vs baseline: 1.5929x; 1.0038x over previous
"""Trainium2 Bass kernel for nn_Block_softmoe (dense transformer block, B=4 S=2048 C=256 H=8).

Strategy (v3)
-------------
Sharding: 8 cores = (batch b, query-half). Each core computes the full block for
1024 query rows of one batch. K/V are computed per-core over that batch's keys
(2x redundant K/V projection; tiny at dim 256). No collectives.

Mask compaction: the key mask (Bernoulli 0/1) is applied on the host by
gathering only the kept key rows (~1024 of 2048), so L ~= 1024, KC = L/128.

Cost-model facts this kernel is built around:
  - matmul cost = out-free-size x cycles_per_row (bf16/f32r>=256: 1.0); the
    stationary (lhsT) load is free -> stream the SMALL operand.
  - only ACT has exp; DVE fakes it with the Schraudolph bit trick
    (int16(a*y+b) bitcast to bf16; int16 saturation gives -0.0 for masked
    keys), so the 64 exp tiles are SPLIT across ACT and DVE (Bresenham
    interleave, ACT-heavy since DVE carries more copy work).
  - every HWDGE dma_start serializes ~625ns on the single HWDGE queue ->
    consolidate input DMAs and push non-critical loads + half the output
    stores through the Pool engine's SWDGE path (Pool is otherwise idle;
    GPSIMD cannot touch PSUM so it only gets SBUF/DRAM work).

Dataflow per core (SQ=1024 queries, L keys):
  QT = WqT.T @ xqT   [256, SQ] f32r (feature-major)   KT likewise [256, L]
  Vone[kc] [128, 8*33] bf16: per head h cols h*33..h*33+31 = V feats, +1 ones
  scores (h,kc): psum[128, SQ] = KT_h[kc].T @ QT_h   (PE, streams queries)
  P[h,kc] = exp(scale*scores + maskbias) -> bf16     (ACT exp | DVE bit trick)
  attnV (qc: 128-query chunk): po[128, 264] += P[h,kc][:,qc].T @ Vone[kc][h]
    -> ONE 33-wide stream gives attn.V AND the softmax denominator (PE).
    NOTE start=True pending-zeroes the whole PSUM bank, so only the first
    write into the po bank sets it.
  normalize: rec = 1/po[:,:,32] (DVE), xout[q,c] = po * rec (DVE, bcast AP)
  transpose xout -> feature-major xoutT via PE identity-transpose
  MLP in 256-col quarter strips (overlaps the attnV tail):
  h1T = gelu(W1T.T @ xoutT + b1) (ACT), final = [h1T;xoutT].T @ [W2T;I]
  (residual fused via identity block), out copies ACT -> DMA (SP/Pool split).

attnV for the first two query chunks is interleaved into the scores phase
(PSUM banks: 4 scores double-buffer + 2 po + 2 proj/mlp = 8).

Self-contained: hardcodes all shapes; compiled NEFF cached per L.
"""

import os
import sys

for _p in ("/opt/trn_rl_repo", "/root/.axon_site/_ro/trn_rl_repo"):
    if os.path.isdir(_p) and _p not in sys.path:
        sys.path.append(_p)

import ml_dtypes
import numpy as np

import concourse.bacc as bacc
import concourse.tile as tile
from concourse import mybir
from concourse.bass_utils import run_bass_kernel_spmd

B, S, C, H, HD = 4, 2048, 256, 8, 32
NCORES = 8
SQ = 1024                      # query rows per core
NQC = SQ // 128                # query chunks for attnV
SCALE = float(HD) ** -0.5
F32 = mybir.dt.float32
F32R = mybir.dt.float32r
BF16 = mybir.dt.bfloat16
I16 = mybir.dt.int16
AF = mybir.ActivationFunctionType
ALU = mybir.AluOpType
NEG = -1e30

# Schraudolph exp in bf16 bits: int16(A*y + B) viewed as bf16 ~= exp(y).
A_EXP = 2.0 ** 7 / np.log(2.0)
B_EXP = 127.0 * 2.0 ** 7 - 4.7
MB2_MASKED = -1e6              # saturates the int16 -> -32768 -> bf16 -0.0

N_DVE_EXP = 29                 # of the 64 exp units, how many go to DVE

_cache: dict = {}


def _build(L: int, use_bv: bool, use_b: bool = True):
    """Build the single-core program (SPMD across 8 cores)."""
    KC = L // 128
    nc = bacc.Bacc("TRN2", target_bir_lowering=False, debug=False, num_devices=NCORES)

    # ---- I/O ----
    d_xqT = nc.dram_tensor("xqT", [C, SQ], BF16, kind="ExternalInput")
    d_xkT = nc.dram_tensor("xkT", [C, L], BF16, kind="ExternalInput")
    d_wqT = nc.dram_tensor("wqT", [C, C], BF16, kind="ExternalInput")
    d_wkT = nc.dram_tensor("wkT", [C, C], BF16, kind="ExternalInput")
    d_wvT = nc.dram_tensor("wvT", [C, C], BF16, kind="ExternalInput")
    d_w1T = nc.dram_tensor("w1T", [C, C], F32R, kind="ExternalInput")
    d_w2TA = nc.dram_tensor("w2TA", [2 * C, C], F32R, kind="ExternalInput")
    d_mbb = nc.dram_tensor("mbb", [128, 2 * KC], F32, kind="ExternalInput")  # mb|mb2
    d_ident = nc.dram_tensor("ident", [128, 128], F32R, kind="ExternalInput")
    d_bqk1 = nc.dram_tensor("bqk1", [128, 6], F32, kind="ExternalInput")  # bq|bk|b1
    d_bvrow = nc.dram_tensor("bvrow", [1, C], F32R, kind="ExternalInput")
    d_out = nc.dram_tensor("out", [SQ, C], F32, kind="ExternalOutput")

    # DVE/ACT exp assignment: Bresenham spread of N_DVE_EXP Ds over 8*KC units
    nu = 8 * KC
    nd = min(N_DVE_EXP, nu)
    dve_units = {u for u in range(nu)
                 if (u * nd) // nu != ((u + 1) * nd) // nu}

    with tile.TileContext(nc) as tc:
        with tc.tile_pool(name="persist", bufs=1) as pp, \
             tc.tile_pool(name="pt", bufs=1) as ptp, \
             tc.tile_pool(name="work", bufs=3) as wp, \
             tc.tile_pool(name="ps_r", bufs=4, space="PSUM") as ps_r:

            # ---- consolidated tiles (chunk-major columns) ----
            xqT = pp.tile([128, 2 * SQ], BF16, tag="xqT", name="xqT")    # cols kk*SQ+q
            xkT = pp.tile([128, 2 * L], BF16, tag="xkT", name="xkT")     # cols kk*L+t
            wqT = pp.tile([128, 2 * C], BF16, tag="wqT", name="wqT")     # cols kk*C+f
            wkT = pp.tile([128, 2 * C], BF16, tag="wkT", name="wkT")
            wvT = pp.tile([128, 2 * C], BF16, tag="wvT", name="wvT")
            w1T = pp.tile([128, 2 * C], F32R, tag="w1T", name="w1T")
            w2TA = pp.tile([128, 4 * C], F32R, tag="w2TA", name="w2TA")  # cols cc*C+f
            mbb = pp.tile([128, 2 * KC], F32, tag="mbb", name="mbb")
            ident = pp.tile([128, 128], F32R, tag="ident", name="ident")

            def _chunks(dram, tile_t, nch, w, dt_):
                # one DMA: DRAM [nch*128, w] -> SBUF [128, nch*w] chunk-major
                nc_ = dram[:, :].rearrange("(c p) w -> p c w", c=nch)
                nc.sync.dma_start(out=tile_t[:, :].rearrange("p (c w) -> p c w", c=nch),
                                  in_=nc_)

            # critical path on HWDGE: weights first (tiny), then x
            _chunks(d_wqT, wqT, 2, C, BF16)
            _chunks(d_wkT, wkT, 2, C, BF16)
            _chunks(d_xqT, xqT, 2, SQ, BF16)
            _chunks(d_xkT, xkT, 2, L, BF16)
            # bulk loads via Pool SWDGE (off the HWDGE queue)
            nc.gpsimd.dma_start(out=mbb, in_=d_mbb[:, :])
            nc.gpsimd.dma_start(out=wvT[:, :].rearrange("p (c w) -> p c w", c=2),
                                in_=d_wvT[:, :].rearrange("(c p) w -> p c w", c=2))
            if use_b:
                bqk1 = pp.tile([128, 6], F32, tag="bqk1", name="bqk1")
                nc.gpsimd.dma_start(out=bqk1, in_=d_bqk1[:, :])
            nc.gpsimd.dma_start(out=ident, in_=d_ident[:, :])
            nc.gpsimd.dma_start(out=w1T[:, :].rearrange("p (c w) -> p c w", c=2),
                                in_=d_w1T[:, :].rearrange("(c p) w -> p c w", c=2))
            nc.gpsimd.dma_start(out=w2TA[:, :].rearrange("p (c w) -> p c w", c=4),
                                in_=d_w2TA[:, :].rearrange("(c p) w -> p c w", c=4))
            if use_bv:
                bvrow = pp.tile([1, C], F32R, tag="bvrow", name="bvrow")
                onesr = pp.tile([1, 128], F32R, tag="onesr", name="onesr")
                nc.gpsimd.dma_start(out=bvrow, in_=d_bvrow[:, :])
                nc.vector.memset(onesr, 1.0)

            mb = mbb[:, 0:KC]
            mb2 = mbb[:, KC:2 * KC]

            # ---- persistent intermediates ----
            QT = [pp.tile([128, SQ], F32R, tag=f"QT{m}", name=f"QT{m}") for m in range(2)]
            KT = [pp.tile([128, L], F32R, tag=f"KT{m}", name=f"KT{m}") for m in range(2)]
            Vone = [pp.tile([128, H * 33], BF16, tag=f"Vone{sc}", name=f"Vone{sc}")
                    for sc in range(KC)]
            # feature-major attn output: cols = cc*SQ + q
            xoutT = pp.tile([128, 2 * SQ], F32R, tag="xoutT", name="xoutT")
            h1T = [pp.tile([128, SQ], F32R, tag=f"h1T{j}", name=f"h1T{j}") for j in range(2)]
            PT = {}

            def _ps():
                # one unified PSUM ring (3 x [128,1024] = 6 banks) shared by
                # proj / scores / transpose / MLP; users slice what they need
                return ps_r.tile([128, SQ], F32, tag="ring", name="ring")

            kchunks = [(o, min(512, L - o)) for o in range(0, L, 512)]

            def emit_qk_proj(m):
                for n in range(2):  # Q: SQ/512
                    pq = _ps()
                    for kk in range(2):
                        nc.tensor.matmul(out=pq[:, 0:512], lhsT=wqT[:, kk * C + m * 128:kk * C + (m + 1) * 128],
                                         rhs=xqT[:, kk * SQ + n * 512:kk * SQ + (n + 1) * 512],
                                         start=(kk == 0), stop=(kk == 1))
                    if use_b:
                        nc.vector.tensor_scalar_add(out=QT[m][:, n * 512:(n + 1) * 512],
                                                    in0=pq[:, 0:512], scalar1=bqk1[:, m:m + 1])
                    else:
                        nc.scalar.copy(out=QT[m][:, n * 512:(n + 1) * 512], in_=pq[:, 0:512])
                for o, w in kchunks:
                    pk = _ps()
                    for kk in range(2):
                        nc.tensor.matmul(out=pk[:, 0:w], lhsT=wkT[:, kk * C + m * 128:kk * C + (m + 1) * 128],
                                         rhs=xkT[:, kk * L + o:kk * L + o + w],
                                         start=(kk == 0), stop=(kk == 1))
                    if use_b:
                        nc.vector.tensor_scalar_add(out=KT[m][:, o:o + w], in0=pk[:, 0:w],
                                                    scalar1=bqk1[:, 2 + m:3 + m])
                    else:
                        nc.vector.tensor_copy(out=KT[m][:, o:o + w], in_=pk[:, 0:w])

            def emit_v_proj(sc):
                pv = _ps()
                for kk in range(2):
                    nc.tensor.matmul(out=pv[:, :C],
                                     lhsT=xkT[:, kk * L + sc * 128:kk * L + (sc + 1) * 128],
                                     rhs=wvT[:, kk * C:(kk + 1) * C], start=(kk == 0),
                                     stop=(kk == 1) and not use_bv)
                if use_bv:
                    nc.tensor.matmul(out=pv[:, :C], lhsT=onesr[0:1, :],
                                     rhs=bvrow[0:1, :], start=False, stop=True)
                vr = Vone[sc][:, :].rearrange("p (h w) -> p h w", h=H)
                nc.vector.tensor_copy(out=vr[:, :, 0:32],
                                      in_=pv[:, :C].rearrange("p (h w) -> p h w", h=H))
                nc.gpsimd.memset(vr[:, :, 32:33], 1.0)

            uidx = [0]

            def emit_score_exp(h, kc):
                g, j = h // 4, h % 4
                pss = _ps()
                for qn in range(2):
                    nc.tensor.matmul(
                        out=pss[:, qn * 512:(qn + 1) * 512],
                        lhsT=KT[g][32 * j:32 * j + 32, kc * 128:(kc + 1) * 128],
                        rhs=QT[g][32 * j:32 * j + 32, qn * 512:(qn + 1) * 512],
                        start=True, stop=True,
                        tile_position=(32 * j, 0))
                pt_t = ptp.tile([128, SQ], BF16, tag="pt", bufs=8 * KC,
                                name=f"pt{h}_{kc}")
                if uidx[0] in dve_units:
                    nc.vector.tensor_scalar(out=pt_t.bitcast(I16), in0=pss,
                                            scalar1=float(SCALE * A_EXP),
                                            scalar2=mb2[:, kc:kc + 1],
                                            op0=ALU.mult, op1=ALU.add)
                else:
                    nc.scalar.activation(out=pt_t, in_=pss, func=AF.Exp,
                                         bias=mb[:, kc:kc + 1], scale=SCALE)
                uidx[0] += 1
                PT[h, kc] = pt_t

            po_of = {}

            def emit_attn(qc, kc):
                if kc == 0:
                    po_of[qc] = _ps()
                po = po_of[qc]
                for h in range(H):
                    # start=True pending-zeroes the WHOLE psum bank, so only
                    # the very first write into the bank may set it.
                    nc.tensor.matmul(
                        out=po[:, h * 33:(h + 1) * 33],
                        lhsT=PT[h, kc][:, qc * 128:(qc + 1) * 128],
                        rhs=Vone[kc][:, h * 33:(h + 1) * 33],
                        start=(kc == 0 and h == 0), stop=(kc == KC - 1),
                        skip_group_check=(h > 0))

            xo_of = {}

            def emit_norm(qc):
                po = po_of[qc][:, 0:H * 33].rearrange("p (h w) -> p h w", h=H)
                rec = wp.tile([128, H, 1], F32, tag="rec", name="rec")
                nc.vector.reciprocal(out=rec, in_=po[:, :, 32:33])
                xo = wp.tile([128, C], F32R, tag="xo", name="xo")
                nc.vector.tensor_mul(out=xo[:, :].rearrange("p (h w) -> p h w", h=H),
                                     in0=po[:, :, 0:32],
                                     in1=rec[:, :, :].broadcast_to((128, H, 32)))
                xo_of[qc] = xo

            def emit_transpose_evac(qc):
                xo = xo_of.pop(qc)
                ptr = _ps()
                for cc in range(2):
                    nc.tensor.transpose(out=ptr[:, cc * 128:(cc + 1) * 128].bitcast(F32R),
                                        in_=xo[:, cc * 128:(cc + 1) * 128],
                                        identity=ident)
                xoT = xoutT[:, :].rearrange("p (c q) -> p c q", c=2)
                nc.vector.tensor_copy(
                    out=xoT[:, :, qc * 128:(qc + 1) * 128],
                    in_=ptr[:, :C].rearrange("p (c q) -> p c q", c=2).bitcast(F32R))

            def emit_mlp_h1(n):
                # 256-col strip: queries n*256..(n+1)*256 (query chunks 2n, 2n+1)
                for j in range(2):
                    ph = _ps()
                    for cc in range(2):
                        nc.tensor.matmul(out=ph[:, 0:256], lhsT=w1T[:, cc * C + j * 128:cc * C + (j + 1) * 128],
                                         rhs=xoutT[:, cc * SQ + n * 256:cc * SQ + (n + 1) * 256],
                                         start=(cc == 0), stop=(cc == 1))
                    nc.scalar.activation(out=h1T[j][:, n * 256:(n + 1) * 256], in_=ph[:, 0:256],
                                         func=AF.Gelu,
                                         bias=(bqk1[:, 4 + j:5 + j] if use_b else 0.0))

            def emit_mlp_final(n):
                for sc in range(2 * n, 2 * n + 2):
                    pf = _ps()
                    for cc in range(4):
                        lh = (h1T[cc][:, sc * 128:(sc + 1) * 128] if cc < 2 else
                              xoutT[:, (cc - 2) * SQ + sc * 128:(cc - 2) * SQ + (sc + 1) * 128])
                        nc.tensor.matmul(out=pf[:, :C], lhsT=lh, rhs=w2TA[:, cc * C:(cc + 1) * C],
                                         start=(cc == 0), stop=(cc == 3))
                    ot = wp.tile([128, C], F32, tag="ot", name="ot")
                    nc.scalar.copy(out=ot, in_=pf[:, :C])
                    if sc % 2 == 0:
                        nc.sync.dma_start(out=d_out[sc * 128:(sc + 1) * 128, :], in_=ot)
                    else:
                        nc.gpsimd.dma_start(out=d_out[sc * 128:(sc + 1) * 128, :], in_=ot)

            # ---- emission schedule ----
            emit_qk_proj(0)
            for h in range(4):
                emit_score_exp(h, 0)
            emit_qk_proj(1)
            for h in range(4, 8):
                emit_score_exp(h, 0)
            for sc in range(min(4, KC)):
                emit_v_proj(sc)
            if KC > 1:
                for h in range(H):
                    emit_score_exp(h, 1)
            for sc in range(4, KC):
                emit_v_proj(sc)
            for kc in range(2, KC):
                for h in range(H):
                    emit_score_exp(h, kc)
            # tail: attnV chains through the same psum ring, with consumers
            # lagged so nothing at the head of the in-order PE queue waits:
            # norm(qc-1) on DVE, transpose+evac(qc-2) on PE (mult long done),
            # MLP h1 for quarter n once evac(2n+1) is emitted, finals one
            # chain later (gelus done by then)
            for qc in range(NQC):
                for kc in range(KC):
                    emit_attn(qc, kc)
                if qc >= 1:
                    emit_norm(qc - 1)
                if qc >= 2:
                    emit_transpose_evac(qc - 2)
                if qc >= 5 and qc % 2 == 1:
                    emit_mlp_h1((qc - 5) // 2)
                if qc >= 6 and qc % 2 == 0:
                    emit_mlp_final((qc - 6) // 2)
            emit_norm(NQC - 1)
            emit_transpose_evac(NQC - 2)
            emit_mlp_final((NQC - 6) // 2)      # quarter 1
            emit_transpose_evac(NQC - 1)
            emit_mlp_h1(NQC // 2 - 2)           # quarter 2
            emit_mlp_final(NQC // 2 - 2)
            emit_mlp_h1(NQC // 2 - 1)           # quarter 3
            emit_mlp_final(NQC // 2 - 1)

    nc.compile()
    return nc


def _prep_inputs(x, mask, Wq, bq, Wk, bk, Wv, bv, W1, b1, W2, b2):
    """Host-side sharding + layout prep. Returns (L, in_maps, use_bv, use_b)."""
    x = np.ascontiguousarray(x, dtype=np.float32)
    keeps = [np.flatnonzero(mask[b, :S] != 0) for b in range(B)]
    cnts = [len(k) for k in keeps]
    L = max(128, -(-max(cnts) // 128) * 128)
    KC = L // 128

    BF = ml_dtypes.bfloat16
    wqT = np.ascontiguousarray(np.asarray(Wq, np.float32).T.astype(BF))
    wkT = np.ascontiguousarray(np.asarray(Wk, np.float32).T.astype(BF))
    wvT = np.ascontiguousarray(np.asarray(Wv, np.float32).T.astype(BF))
    w1T = np.ascontiguousarray(W1.T, dtype=np.float32)
    w2TA = np.ascontiguousarray(
        np.vstack([W2.T.astype(np.float32), np.eye(C, dtype=np.float32)]))
    ident = np.eye(128, dtype=np.float32)
    bqk1 = np.stack([
        bq[0:128], bq[128:256], bk[0:128], bk[128:256], b1[0:128], b1[128:256],
    ], axis=1).astype(np.float32)
    bvrow = np.asarray(bv, np.float32).reshape(1, C)
    use_bv = bool(np.any(bv != 0))
    use_b = bool(np.any(bq != 0) or np.any(bk != 0) or np.any(b1 != 0))

    in_maps = []
    for core in range(NCORES):
        b, half = core // 2, core % 2
        xb = x[b]                                   # [S, C]
        xqT = np.ascontiguousarray(xb[half * SQ:(half + 1) * SQ].T.astype(BF))  # [C, SQ]
        xk = np.zeros((L, C), dtype=np.float32)
        xk[:cnts[b]] = xb[keeps[b]]
        xkT = np.ascontiguousarray(xk.T.astype(BF))  # [C, L]
        mb1d = np.full(L, NEG, dtype=np.float32)
        mb1d[:cnts[b]] = 0.0
        mb = mb1d.reshape(KC, 128).T                 # [128, KC]
        mb2_1d = np.full(L, MB2_MASKED, dtype=np.float32)
        mb2_1d[:cnts[b]] = B_EXP
        mb2 = mb2_1d.reshape(KC, 128).T              # [128, KC]
        mbb = np.ascontiguousarray(np.concatenate([mb, mb2], axis=1))  # [128, 2KC]
        in_maps.append({
            "xqT": xqT, "xkT": xkT, "wqT": wqT, "wkT": wkT, "wvT": wvT,
            "w1T": w1T, "w2TA": w2TA, "mbb": mbb, "ident": ident,
            "bqk1": bqk1, "bvrow": bvrow,
        })
    return L, in_maps, use_bv, use_b


def kernel(x, mask, Wq, bq, Wk, bk, Wv, bv, W1, b1, W2, b2):
    L, in_maps, use_bv, use_b = _prep_inputs(x, mask, Wq, bq, Wk, bk, Wv, bv, W1, b1, W2, b2)
    key = (L, use_bv, use_b)
    if key not in _cache:
        _cache[key] = _build(L, use_bv, use_b)
    nc = _cache[key]
    res = None
    last_exc = None
    for attempt in range(4):
        try:
            res = run_bass_kernel_spmd(nc, in_maps, core_ids=list(range(NCORES)),
                                       trace=False)
            break
        except Exception as e:  # transient device errors on first exec of a NEFF
            last_exc = e
            import time as _time
            import jax as _jax
            _time.sleep(2.0)
            try:
                _jax.clear_caches()
            except Exception:
                pass
    if res is None:
        raise last_exc
    out = np.empty((B, S, C), dtype=np.float32)
    for core in range(NCORES):
        b, half = core // 2, core % 2
        out[b, half * SQ:(half + 1) * SQ] = res.results[core]["out"]
    if np.any(b2 != 0):
        out += np.asarray(b2, dtype=np.float32)[None, None, :]
    # stash for test harness reuse (timing reruns)
    kernel.last = {"nc": nc, "in_maps": in_maps, "L": L}
    return out
